# revision 6
# baseline (speedup 1.0000x reference)
"""BrainGCN Trainium2 kernel (8 NeuronCores, Bass/Tile).

Model (PyG-style GCNConv x2 + 2 FC layers):
    h = tanh(gcn(x,  W1, b1)); h = tanh(gcn(h, W2, b2))
    h = tanh(h @ W3 + b3);      out = h @ W4 + b4

gcn(x, W, b) = (agg + x * dinv^2) @ W + b  with
    agg[d] = sum_{e:(s,d)} dinv[s]*dinv[d] * x[s]        (by linearity we
aggregate raw feature rows first, then apply W once per node).

Distribution: dst-nodes are split into 8 contiguous blocks (one per core).
Each core aggregates its own dst block; self-loops are folded in as ordinary
edges with coef = dinv^2.  The only cross-core exchange is an AllGather of
the h1 shards between the two GCN layers.

Device-side scatter-add: edges are sorted by dst into 128-wide dst windows.
For every 128-edge tile the DVE builds S[e, d_local] = coef[e] *
onehot(d_local[e]) with a single tensor_scalar (iota == dloc) * coef, and the
PE accumulates aggT[feat, dst] += E_tile^T @ S into PSUM.  Gathers of the
512-byte feature rows run on dma_gather (int16 indices -> the table is
addressed in halves of 25000 rows).
"""

import math

import numpy as np

# ---------------------------------------------------------------- constants
N_NODES = 50000
N_CORES = 8
F_IN, H1D, H2D, H3D, OUTD = 128, 128, 64, 64, 1
WIN = 128          # dst window width (psum free dim of the scatter matmul)
HALF = 25000       # gather-table half size (int16 index range)
G_WINDOWS = 6      # dst windows per gather chunk (per half)
NCHUNK = 512       # fc-layer column chunk


def _cdiv(a, b):
    return -(-a // b)


def _rup(a, b):
    return _cdiv(a, b) * b


# ------------------------------------------------------------------ planning
class Plan:
    pass


def make_plan(edge_index, n_nodes=N_NODES, n_cores=N_CORES, half=HALF,
              g_windows=G_WINDOWS, win=WIN):
    """Host-side graph preprocessing -> static schedule + per-core arrays."""
    src = np.asarray(edge_index[0]).astype(np.int64)
    dst = np.asarray(edge_index[1]).astype(np.int64)

    npc = n_nodes // n_cores
    assert npc * n_cores == n_nodes
    n_win = _cdiv(npc, win)
    n_half = _cdiv(n_nodes, half)
    assert half <= 32767

    deg = np.bincount(dst, minlength=n_nodes).astype(np.float64) + 1.0
    dinv = 1.0 / np.sqrt(deg)

    # append self loops as ordinary edges
    loops = np.arange(n_nodes, dtype=np.int64)
    s_all = np.concatenate([src, loops])
    d_all = np.concatenate([dst, loops])
    coef = (dinv[s_all] * dinv[d_all]).astype(np.float32)

    core = d_all // npc
    w = (d_all % npc) // win
    h = s_all // half

    # per (core, w, h) counts -> static caps shared by all cores
    gid = (core * n_win + w) * n_half + h
    counts = np.bincount(gid, minlength=n_cores * n_win * n_half)
    counts = counts.reshape(n_cores, n_win, n_half)
    caps = _rup(counts.max(axis=0), 128)          # [n_win, n_half] slots

    # window groups (gather chunks)
    wgroups = [list(range(i, min(i + g_windows, n_win)))
               for i in range(0, n_win, g_windows)]

    # static slot offsets, in [wg][h][w] order
    off = {}
    calls = []        # (wg_index, h, slot_off, n_slots)
    pos = 0
    for gi, wg in enumerate(wgroups):
        for hh in range(n_half):
            call_off = pos
            for ww in wg:
                off[(ww, hh)] = pos
                pos += int(caps[ww, hh])
            calls.append((gi, hh, call_off, pos - call_off))
    S = pos                                        # total slots (mult of 128)
    assert S % 128 == 0
    T = S // 128                                   # total tiles

    # per-window tile lists: [(h, tile_local_in_call, tile_global)]
    win_tiles = []
    for ww in range(n_win):
        tiles = []
        for hh in range(n_half):
            gi = ww // g_windows
            call_off = next(c[2] for c in calls if c[0] == gi and c[1] == hh)
            o = off[(ww, hh)]
            for t in range(int(caps[ww, hh]) // 128):
                tiles.append((hh, (o - call_off) // 128 + t, o // 128 + t))
        win_tiles.append(tiles)

    # ------- per-core arrays
    # order edges: core, then (wg, h, w) -- same as the slot layout
    wg_of_w = np.array([ww // g_windows for ww in range(n_win)])
    order_key = (((core * len(wgroups) + wg_of_w[w]) * n_half + h) * n_win + w)
    order = np.argsort(order_key, kind="stable")
    s_o, d_o, c_o = s_all[order], d_all[order], coef[order]
    core_o, w_o, h_o = core[order], w[order], h[order]

    # destination slot for each edge: static group offset + rank within group.
    # each (core, w, h) group is contiguous in the sorted order; rank = index
    # since the group's first element.
    comb = (core_o * n_win + w_o) * n_half + h_o
    pos = np.arange(comb.size, dtype=np.int64)
    is_start = np.ones(comb.size, dtype=bool)
    if comb.size > 1:
        is_start[1:] = comb[1:] != comb[:-1]
    rank = pos - np.maximum.accumulate(np.where(is_start, pos, 0))

    static_off = np.zeros((n_win, n_half), dtype=np.int64)
    for ww in range(n_win):
        for hh in range(n_half):
            static_off[ww, hh] = off[(ww, hh)]
    slot = static_off[w_o, h_o] + rank

    idx16 = np.zeros((n_cores, S), dtype=np.int16)
    dloc = np.zeros((n_cores, S), dtype=np.float32)
    cof = np.zeros((n_cores, S), dtype=np.float32)
    ci = core_o.astype(np.int64)
    idx16[ci, slot] = (s_o - h_o * half).astype(np.int16)
    dloc[ci, slot] = (d_o - ci * npc - w_o * win).astype(np.float32)
    cof[ci, slot] = c_o

    p = Plan()
    p.n_nodes, p.n_cores, p.npc = n_nodes, n_cores, npc
    p.win, p.n_win, p.half, p.n_half = win, n_win, half, n_half
    p.win_sizes = [min(win, npc - ww * win) for ww in range(n_win)]
    p.wgroups, p.calls, p.win_tiles = wgroups, calls, win_tiles
    p.S, p.T = S, T
    # layouts for the device: idx [128, S/16] (16-row wrap, replicated x8),
    # dloc/coef [128, T] (slot i -> [i%128, i//128])
    p.idx_arr = np.ascontiguousarray(
        np.tile(idx16.reshape(n_cores, S // 16, 16).transpose(0, 2, 1),
                (1, 8, 1)))
    p.dloc_arr = np.ascontiguousarray(dloc.reshape(n_cores, T, 128)
                                      .transpose(0, 2, 1))
    p.coef_arr = np.ascontiguousarray(cof.reshape(n_cores, T, 128)
                                      .transpose(0, 2, 1))
    return p


# ------------------------------------------------------------------- program
def build_program(p, debug=False):
    import concourse.bacc as bacc
    import concourse.bass as bass
    import concourse.mybir as mybir
    import concourse.tile as tile
    from concourse.masks import make_identity

    f32 = mybir.dt.float32
    i16 = mybir.dt.int16
    AF = mybir.ActivationFunctionType
    OP = mybir.AluOpType

    nc = bacc.Bacc("TRN2", target_bir_lowering=False, debug=debug,
                   num_devices=p.n_cores)

    x_d = nc.dram_tensor("x", [p.n_nodes, F_IN], f32, kind="ExternalInput")
    idx_d = nc.dram_tensor("midx", [128, p.S // 16], i16, kind="ExternalInput")
    dloc_d = nc.dram_tensor("mdloc", [128, p.T], f32, kind="ExternalInput")
    coef_d = nc.dram_tensor("mcoef", [128, p.T], f32, kind="ExternalInput")
    iota_d = nc.dram_tensor("iota", [128, p.win], f32, kind="ExternalInput")
    w1_d = nc.dram_tensor("w1", [F_IN, H1D], f32, kind="ExternalInput")
    b1_d = nc.dram_tensor("b1", [H1D, 1], f32, kind="ExternalInput")
    w2_d = nc.dram_tensor("w2", [H1D, H2D], f32, kind="ExternalInput")
    b2_d = nc.dram_tensor("b2", [H2D, 1], f32, kind="ExternalInput")
    w3_d = nc.dram_tensor("w3", [H2D, H3D], f32, kind="ExternalInput")
    b3_d = nc.dram_tensor("b3", [H3D, 1], f32, kind="ExternalInput")
    w4_d = nc.dram_tensor("w4", [H3D, OUTD], f32, kind="ExternalInput")
    b4_d = nc.dram_tensor("b4", [OUTD, 1], f32, kind="ExternalInput")
    out_d = nc.dram_tensor("out", [p.npc, OUTD], f32, kind="ExternalOutput")

    h1_shard = nc.dram_tensor("h1_shard", [p.npc, H1D], f32)
    h1_full = nc.dram_tensor("h1_full", [p.n_nodes, H1D], f32,
                             addr_space="Shared")

    with tile.TileContext(nc) as tc:
        with (
            tc.tile_pool(name="const", bufs=1) as cpool,
            tc.tile_pool(name="gather", bufs=2) as gpool,
            tc.tile_pool(name="sel", bufs=4) as spool,
            tc.tile_pool(name="work", bufs=3) as wpool,
            tc.tile_pool(name="persist", bufs=1) as ppool,
            tc.tile_pool(name="psum", bufs=2, space="PSUM") as pspool,
            tc.tile_pool(name="psumfc", bufs=1, space="PSUM") as pfpool,
        ):
            # ---- constants / metadata to SBUF
            idx_s = cpool.tile([128, p.S // 16], i16)
            nc.sync.dma_start(idx_s[:], idx_d[:, :])
            dloc_s = cpool.tile([128, p.T], f32)
            nc.sync.dma_start(dloc_s[:], dloc_d[:, :])
            coef_s = cpool.tile([128, p.T], f32)
            nc.sync.dma_start(coef_s[:], coef_d[:, :])
            iota_s = cpool.tile([128, p.win], f32)
            nc.sync.dma_start(iota_s[:], iota_d[:, :])
            w1_s = cpool.tile([F_IN, H1D], f32)
            nc.sync.dma_start(w1_s[:], w1_d[:, :])
            b1_s = cpool.tile([H1D, 1], f32)
            nc.sync.dma_start(b1_s[:], b1_d[:, :])
            w2_s = cpool.tile([H1D, H2D], f32)
            nc.sync.dma_start(w2_s[:], w2_d[:, :])
            b2_s = cpool.tile([H2D, 1], f32)
            nc.sync.dma_start(b2_s[:], b2_d[:, :])
            w3_s = cpool.tile([H2D, H3D], f32)
            nc.sync.dma_start(w3_s[:], w3_d[:, :])
            b3_s = cpool.tile([H3D, 1], f32)
            nc.sync.dma_start(b3_s[:], b3_d[:, :])
            w4_s = cpool.tile([H3D, OUTD], f32)
            nc.sync.dma_start(w4_s[:], w4_d[:, :])
            b4_s = cpool.tile([OUTD, 1], f32)
            nc.sync.dma_start(b4_s[:], b4_d[:, :])
            ident = cpool.tile([128, 128], f32)
            make_identity(nc, ident[:])

            h2T = ppool.tile([H2D, p.npc], f32)

            # ---------------- one GCN layer ----------------
            def gcn_layer(table_ap_fn, w_s, b_s, out_feat, sink):
                for gi, wg in enumerate(p.wgroups):
                    bufs = {}
                    for (cgi, hh, call_off, n_call) in p.calls:
                        if cgi != gi or n_call == 0:
                            continue
                        gb = gpool.tile([128, (n_call // 128) * F_IN], f32,
                                        tag=f"gb{hh}")
                        out3d = gb[:].rearrange("q (t e) -> q t e", e=F_IN)
                        nc.gpsimd.dma_gather(
                            out_ap=out3d,
                            in_ap=table_ap_fn(hh),
                            idxs_ap=idx_s[:, call_off // 16:
                                          (call_off + n_call) // 16],
                            num_idxs=n_call,
                            num_idxs_reg=n_call,
                            elem_size=F_IN,
                            single_packet=False,
                        )
                        bufs[hh] = gb
                    for ww in wg:
                        tiles = p.win_tiles[ww]
                        if not tiles:
                            continue
                        wsz = p.win_sizes[ww]
                        pag = pspool.tile([128, p.win], f32, tag="pag")
                        for k, (hh, lt, gt) in enumerate(tiles):
                            st = spool.tile([128, p.win], f32, tag="st")
                            nc.vector.tensor_scalar(
                                out=st[:], in0=iota_s[:],
                                scalar1=dloc_s[:, gt:gt + 1],
                                scalar2=coef_s[:, gt:gt + 1],
                                op0=OP.is_equal, op1=OP.mult)
                            nc.tensor.matmul(
                                pag[:],
                                lhsT=bufs[hh][:, lt * F_IN:(lt + 1) * F_IN],
                                rhs=st[:],
                                start=(k == 0), stop=(k == len(tiles) - 1))
                        aggT = wpool.tile([128, p.win], f32, tag="aggT")
                        nc.vector.tensor_copy(aggT[:], pag[:])
                        ph = pspool.tile([out_feat, p.win], f32, tag="ph")
                        nc.tensor.matmul(ph[:], lhsT=w_s[:], rhs=aggT[:],
                                         start=True, stop=True)
                        sink(ww, wsz, ph, b_s)

            # ---- layer 1: x -> h1_shard (node-major, via PE transpose)
            def sink1(ww, wsz, ph, b_s):
                hT = wpool.tile([128, p.win], f32, tag="hT")
                nc.scalar.activation(hT[:], ph[:], AF.Tanh,
                                     bias=b_s[:, 0:1])
                pt = pspool.tile([128, 128], f32, tag="pt")
                nc.tensor.transpose(pt[:], hT[:], ident[:])
                hw_ = wpool.tile([128, 128], f32, tag="hw")
                nc.vector.tensor_copy(hw_[:], pt[:])
                nc.sync.dma_start(
                    h1_shard[ww * p.win: ww * p.win + wsz, :],
                    hw_[:wsz, :])

            gcn_layer(lambda hh: x_d[hh * p.half:
                                     min((hh + 1) * p.half, p.n_nodes), :],
                      w1_s, b1_s, H1D, sink1)

            # ---- exchange h1 shards
            nc.gpsimd.collective_compute(
                "AllGather", mybir.AluOpType.bypass,
                replica_groups=[list(range(p.n_cores))],
                ins=[h1_shard[:, :]], outs=[h1_full[:, :]])

            # ---- layer 2: h1_full -> h2T (kept on-chip, feat-major)
            def sink2(ww, wsz, ph, b_s):
                nc.scalar.activation(
                    h2T[:, ww * p.win: ww * p.win + wsz],
                    ph[:, :wsz], AF.Tanh, bias=b_s[:, 0:1])

            gcn_layer(lambda hh: h1_full[hh * p.half:
                                         min((hh + 1) * p.half, p.n_nodes), :],
                      w2_s, b2_s, H2D, sink2)

            # ---- fc layers on the dst shard
            for c0 in range(0, p.npc, NCHUNK):
                cs = min(NCHUNK, p.npc - c0)
                p3 = pfpool.tile([H3D, NCHUNK], f32, tag="p3")
                nc.tensor.matmul(p3[:, :cs], lhsT=w3_s[:],
                                 rhs=h2T[:, c0:c0 + cs],
                                 start=True, stop=True)
                h3 = wpool.tile([H3D, NCHUNK], f32, tag="h3")
                nc.scalar.activation(h3[:, :cs], p3[:, :cs], AF.Tanh,
                                     bias=b3_s[:, 0:1])
                p4 = pfpool.tile([OUTD, NCHUNK], f32, tag="p4")
                nc.tensor.matmul(p4[:, :cs], lhsT=w4_s[:], rhs=h3[:, :cs],
                                 start=True, stop=True)
                ob = wpool.tile([OUTD, NCHUNK], f32, tag="ob")
                nc.vector.tensor_scalar(
                    out=ob[:, :cs], in0=p4[:, :cs],
                    scalar1=b4_s[0:1, 0:1], scalar2=None, op0=OP.add)
                nc.sync.dma_start(out_d[c0:c0 + cs, :], ob[0:1, :cs])

    nc.compile()
    return nc


def make_in_maps(p, inputs):
    x = np.ascontiguousarray(np.asarray(inputs["x"], dtype=np.float32))
    iota = np.tile(np.arange(p.win, dtype=np.float32)[None, :], (128, 1))
    maps = []
    for c in range(p.n_cores):
        maps.append({
            "x": x,
            "midx": p.idx_arr[c],
            "mdloc": p.dloc_arr[c],
            "mcoef": p.coef_arr[c],
            "iota": iota,
            "w1": np.asarray(inputs["W1"], dtype=np.float32),
            "b1": np.asarray(inputs["b1"], dtype=np.float32).reshape(-1, 1),
            "w2": np.asarray(inputs["W2"], dtype=np.float32),
            "b2": np.asarray(inputs["b2"], dtype=np.float32).reshape(-1, 1),
            "w3": np.asarray(inputs["W3"], dtype=np.float32),
            "b3": np.asarray(inputs["b3"], dtype=np.float32).reshape(-1, 1),
            "w4": np.asarray(inputs["W4"], dtype=np.float32),
            "b4": np.asarray(inputs["b4"], dtype=np.float32).reshape(-1, 1),
        })
    return maps


_CACHE = {}


def kernel(_trace=False, **inputs):
    from concourse.bass_utils import run_bass_kernel_spmd

    edge_index = np.asarray(inputs["edge_index"])
    p = make_plan(edge_index)
    key = (p.S, tuple(int(c[3]) for c in p.calls))
    if key not in _CACHE:
        _CACHE[key] = build_program(p)
    nc = _CACHE[key]
    res = run_bass_kernel_spmd(nc, make_in_maps(p, inputs),
                               core_ids=list(range(p.n_cores)),
                               trace=_trace)
    out = np.concatenate([res.results[c]["out"] for c in range(p.n_cores)],
                         axis=0)
    if _trace:
        return out, res
    return out


# revision 12
# speedup vs baseline: 1.0303x; 1.0303x over previous
"""BrainGCN Trainium2 kernel (8 NeuronCores, Bass/Tile).

Model (PyG-style GCNConv x2 + 2 FC layers):
    h = tanh(gcn(x,  W1, b1)); h = tanh(gcn(h, W2, b2))
    h = tanh(h @ W3 + b3);      out = h @ W4 + b4

gcn(x, W, b) = (agg + x * dinv^2) @ W + b  with
    agg[d] = sum_{e:(s,d)} dinv[s]*dinv[d] * x[s]        (by linearity we
aggregate raw feature rows first, then apply W once per node).

Distribution: dst-nodes are split into 8 contiguous blocks (one per core).
Each core aggregates its own dst block; self-loops are folded in as ordinary
edges with coef = dinv^2.  The only cross-core exchange is an AllGather of
the h1 shards between the two GCN layers.

Device-side scatter-add: edges are sorted by dst into 128-wide dst windows.
For every 128-edge tile the DVE builds S[e, d_local] = coef[e] *
onehot(d_local[e]) with a single tensor_scalar (iota == dloc) * coef, and the
PE accumulates aggT[feat, dst] += E_tile^T @ S into PSUM.  Gathers of the
512-byte feature rows run on dma_gather (int16 indices -> the table is
addressed in halves of 25000 rows).
"""

import math

import numpy as np

# ---------------------------------------------------------------- constants
N_NODES = 50000
N_CORES = 8
F_IN, H1D, H2D, H3D, OUTD = 128, 128, 64, 64, 1
WIN = 128          # dst window width (psum free dim of the scatter matmul)
HALF = 25000       # gather-table half size (int16 index range)
G_WINDOWS = 6      # dst windows per gather chunk (per half)
NCHUNK = 512       # fc-layer column chunk


def _cdiv(a, b):
    return -(-a // b)


def _rup(a, b):
    return _cdiv(a, b) * b


# ------------------------------------------------------------------ planning
class Plan:
    pass


def make_plan(edge_index, n_nodes=N_NODES, n_cores=N_CORES, half=HALF,
              g_windows=G_WINDOWS, win=WIN):
    """Host-side graph preprocessing -> static schedule + per-core arrays."""
    src = np.asarray(edge_index[0]).astype(np.int64)
    dst = np.asarray(edge_index[1]).astype(np.int64)

    npc = n_nodes // n_cores
    assert npc * n_cores == n_nodes
    n_win = _cdiv(npc, win)
    n_half = _cdiv(n_nodes, half)
    assert half <= 32767

    deg = np.bincount(dst, minlength=n_nodes).astype(np.float64) + 1.0
    dinv = 1.0 / np.sqrt(deg)

    # append self loops as ordinary edges
    loops = np.arange(n_nodes, dtype=np.int64)
    s_all = np.concatenate([src, loops])
    d_all = np.concatenate([dst, loops])
    coef = (dinv[s_all] * dinv[d_all]).astype(np.float32)

    core = d_all // npc
    w = (d_all % npc) // win
    h = s_all // half

    # per (core, w, h) counts -> static caps shared by all cores
    gid = (core * n_win + w) * n_half + h
    counts = np.bincount(gid, minlength=n_cores * n_win * n_half)
    counts = counts.reshape(n_cores, n_win, n_half)
    caps = _rup(counts.max(axis=0), 128)          # [n_win, n_half] slots

    # window groups (gather chunks)
    wgroups = [list(range(i, min(i + g_windows, n_win)))
               for i in range(0, n_win, g_windows)]

    # static slot offsets, in [wg][h][w] order
    off = {}
    calls = []        # (wg_index, h, slot_off, n_slots)
    pos = 0
    for gi, wg in enumerate(wgroups):
        for hh in range(n_half):
            call_off = pos
            for ww in wg:
                off[(ww, hh)] = pos
                pos += int(caps[ww, hh])
            calls.append((gi, hh, call_off, pos - call_off))
    S = pos                                        # total slots (mult of 128)
    assert S % 128 == 0
    T = S // 128                                   # total tiles

    # per-window tile lists: [(h, tile_local_in_call, tile_global)]
    win_tiles = []
    for ww in range(n_win):
        tiles = []
        for hh in range(n_half):
            gi = ww // g_windows
            call_off = next(c[2] for c in calls if c[0] == gi and c[1] == hh)
            o = off[(ww, hh)]
            for t in range(int(caps[ww, hh]) // 128):
                tiles.append((hh, (o - call_off) // 128 + t, o // 128 + t))
        win_tiles.append(tiles)

    # ------- per-core arrays
    # order edges: core, then (wg, h, w) -- same as the slot layout
    wg_of_w = np.array([ww // g_windows for ww in range(n_win)])
    order_key = (((core * len(wgroups) + wg_of_w[w]) * n_half + h) * n_win + w)
    order = np.argsort(order_key, kind="stable")
    s_o, d_o, c_o = s_all[order], d_all[order], coef[order]
    core_o, w_o, h_o = core[order], w[order], h[order]

    # destination slot for each edge: static group offset + rank within group.
    # each (core, w, h) group is contiguous in the sorted order; rank = index
    # since the group's first element.
    comb = (core_o * n_win + w_o) * n_half + h_o
    pos = np.arange(comb.size, dtype=np.int64)
    is_start = np.ones(comb.size, dtype=bool)
    if comb.size > 1:
        is_start[1:] = comb[1:] != comb[:-1]
    rank = pos - np.maximum.accumulate(np.where(is_start, pos, 0))

    static_off = np.zeros((n_win, n_half), dtype=np.int64)
    for ww in range(n_win):
        for hh in range(n_half):
            static_off[ww, hh] = off[(ww, hh)]
    slot = static_off[w_o, h_o] + rank

    idx16 = np.zeros((n_cores, S), dtype=np.int16)
    dloc = np.zeros((n_cores, S), dtype=np.float32)
    cof = np.zeros((n_cores, S), dtype=np.float32)
    ci = core_o.astype(np.int64)
    idx16[ci, slot] = (s_o - h_o * half).astype(np.int16)
    dloc[ci, slot] = (d_o - ci * npc - w_o * win).astype(np.float32)
    cof[ci, slot] = c_o

    p = Plan()
    p.n_nodes, p.n_cores, p.npc = n_nodes, n_cores, npc
    p.win, p.n_win, p.half, p.n_half = win, n_win, half, n_half
    p.win_sizes = [min(win, npc - ww * win) for ww in range(n_win)]
    p.wgroups, p.calls, p.win_tiles = wgroups, calls, win_tiles
    p.S, p.T = S, T
    # layouts for the device: idx [128, S/16] (16-row wrap, replicated x8),
    # dloc/coef [128, T] (slot i -> [i%128, i//128])
    p.idx_arr = np.ascontiguousarray(
        np.tile(idx16.reshape(n_cores, S // 16, 16).transpose(0, 2, 1),
                (1, 8, 1)))
    p.dloc_arr = np.ascontiguousarray(dloc.reshape(n_cores, T, 128)
                                      .transpose(0, 2, 1))
    p.coef_arr = np.ascontiguousarray(cof.reshape(n_cores, T, 128)
                                      .transpose(0, 2, 1))
    return p


# ------------------------------------------------------------------- program
def build_program(p, debug=False, n_queues=4, scratch=32768, f32r=False):
    import concourse.bacc as bacc
    import concourse.bass as bass
    import concourse.mybir as mybir
    import concourse.tile as tile
    from concourse.masks import make_identity

    f32 = mybir.dt.float32
    f32r = mybir.dt.float32r if f32r else f32
    i16 = mybir.dt.int16
    AF = mybir.ActivationFunctionType
    OP = mybir.AluOpType

    nc = bacc.Bacc("TRN2", target_bir_lowering=False, debug=debug,
                   num_devices=p.n_cores, num_swdge_queues=n_queues,
                   dynamic_dma_scratch_size=scratch)

    x_d = nc.dram_tensor("x", [p.n_nodes, F_IN], f32, kind="ExternalInput")
    idx_d = nc.dram_tensor("midx", [128, p.S // 16], i16, kind="ExternalInput")
    dloc_d = nc.dram_tensor("mdloc", [128, p.T], f32, kind="ExternalInput")
    coef_d = nc.dram_tensor("mcoef", [128, p.T], f32, kind="ExternalInput")
    iota_d = nc.dram_tensor("iota", [128, p.win], f32, kind="ExternalInput")
    w1_d = nc.dram_tensor("w1", [F_IN, H1D], f32, kind="ExternalInput")
    b1_d = nc.dram_tensor("b1", [H1D, 1], f32, kind="ExternalInput")
    w2_d = nc.dram_tensor("w2", [H1D, H2D], f32, kind="ExternalInput")
    b2_d = nc.dram_tensor("b2", [H2D, 1], f32, kind="ExternalInput")
    w3_d = nc.dram_tensor("w3", [H2D, H3D], f32, kind="ExternalInput")
    b3_d = nc.dram_tensor("b3", [H3D, 1], f32, kind="ExternalInput")
    w4_d = nc.dram_tensor("w4", [H3D, OUTD], f32, kind="ExternalInput")
    b4_d = nc.dram_tensor("b4", [OUTD, 1], f32, kind="ExternalInput")
    out_d = nc.dram_tensor("out", [p.npc, OUTD], f32, kind="ExternalOutput")

    h1_shard = nc.dram_tensor("h1_shard", [p.npc, H1D], f32)
    h1_full = nc.dram_tensor("h1_full", [p.n_nodes, H1D], f32,
                             addr_space="Shared")

    with tile.TileContext(nc) as tc:
        with (
            tc.tile_pool(name="const", bufs=1) as cpool,
            tc.tile_pool(name="gather", bufs=2) as gpool,
            tc.tile_pool(name="sel", bufs=4) as spool,
            tc.tile_pool(name="work", bufs=3) as wpool,
            tc.tile_pool(name="persist", bufs=1) as ppool,
            tc.tile_pool(name="psum", bufs=2, space="PSUM") as pspool,
            tc.tile_pool(name="psumfc", bufs=1, space="PSUM") as pfpool,
        ):
            # ---- constants / metadata to SBUF
            idx_s = cpool.tile([128, p.S // 16], i16)
            nc.sync.dma_start(idx_s[:], idx_d[:, :])
            dloc_s = cpool.tile([128, p.T], f32)
            nc.sync.dma_start(dloc_s[:], dloc_d[:, :])
            coef_s = cpool.tile([128, p.T], f32)
            nc.sync.dma_start(coef_s[:], coef_d[:, :])
            iota_s = cpool.tile([128, p.win], f32)
            nc.sync.dma_start(iota_s[:], iota_d[:, :])
            w1_s = cpool.tile([F_IN, H1D], f32)
            nc.sync.dma_start(w1_s[:], w1_d[:, :])
            b1_s = cpool.tile([H1D, 1], f32)
            nc.sync.dma_start(b1_s[:], b1_d[:, :])
            w2_s = cpool.tile([H1D, H2D], f32)
            nc.sync.dma_start(w2_s[:], w2_d[:, :])
            b2_s = cpool.tile([H2D, 1], f32)
            nc.sync.dma_start(b2_s[:], b2_d[:, :])
            w3_s = cpool.tile([H2D, H3D], f32)
            nc.sync.dma_start(w3_s[:], w3_d[:, :])
            b3_s = cpool.tile([H3D, 1], f32)
            nc.sync.dma_start(b3_s[:], b3_d[:, :])
            w4_s = cpool.tile([H3D, OUTD], f32)
            nc.sync.dma_start(w4_s[:], w4_d[:, :])
            b4_s = cpool.tile([OUTD, 1], f32)
            nc.sync.dma_start(b4_s[:], b4_d[:, :])
            ident = cpool.tile([128, 128], f32)
            make_identity(nc, ident[:])

            h2T = ppool.tile([H2D, p.npc], f32)

            # ---------------- one GCN layer ----------------
            call_seq = [0]

            def gcn_layer(table_ap_fn, w_s, b_s, out_feat, sink):
                for gi, wg in enumerate(p.wgroups):
                    bufs = {}
                    for (cgi, hh, call_off, n_call) in p.calls:
                        if cgi != gi or n_call == 0:
                            continue
                        gb = gpool.tile([128, (n_call // 128) * F_IN], f32,
                                        tag=f"gb{hh}")
                        out3d = gb[:].rearrange("q (t e) -> q t e", e=F_IN)
                        nc.gpsimd.dma_gather(
                            out_ap=out3d,
                            in_ap=table_ap_fn(hh),
                            idxs_ap=idx_s[:, call_off // 16:
                                          (call_off + n_call) // 16],
                            num_idxs=n_call,
                            num_idxs_reg=n_call,
                            elem_size=F_IN,
                            single_packet=False,
                            queue_num=call_seq[0] % n_queues,
                        )
                        call_seq[0] += 1
                        bufs[hh] = gb
                    for ww in wg:
                        tiles = p.win_tiles[ww]
                        if not tiles:
                            continue
                        wsz = p.win_sizes[ww]
                        pag = pspool.tile([128, p.win], f32, tag="pag")
                        for k, (hh, lt, gt) in enumerate(tiles):
                            st = spool.tile([128, p.win], f32, tag="st")
                            nc.vector.tensor_scalar(
                                out=st[:], in0=iota_s[:],
                                scalar1=dloc_s[:, gt:gt + 1],
                                scalar2=coef_s[:, gt:gt + 1],
                                op0=OP.is_equal, op1=OP.mult)
                            nc.tensor.matmul(
                                pag[:],
                                lhsT=bufs[hh][:, lt * F_IN:(lt + 1) * F_IN]
                                .bitcast(f32r),
                                rhs=st[:].bitcast(f32r),
                                start=(k == 0), stop=(k == len(tiles) - 1))
                        aggT = wpool.tile([128, p.win], f32, tag="aggT")
                        nc.vector.tensor_copy(aggT[:], pag[:])
                        ph = pspool.tile([out_feat, p.win], f32, tag="ph")
                        nc.tensor.matmul(ph[:], lhsT=w_s[:].bitcast(f32r),
                                         rhs=aggT[:].bitcast(f32r),
                                         start=True, stop=True)
                        sink(ww, wsz, ph, b_s)

            # ---- layer 1: x -> h1_shard (node-major, via PE transpose)
            def sink1(ww, wsz, ph, b_s):
                hT = wpool.tile([128, p.win], f32, tag="hT")
                nc.scalar.activation(hT[:], ph[:], AF.Tanh,
                                     bias=b_s[:, 0:1])
                pt = pspool.tile([128, 128], f32, tag="pt")
                nc.tensor.transpose(pt[:], hT[:], ident[:])
                hw_ = wpool.tile([128, 128], f32, tag="hw")
                nc.vector.tensor_copy(hw_[:], pt[:])
                nc.sync.dma_start(
                    h1_shard[ww * p.win: ww * p.win + wsz, :],
                    hw_[:wsz, :])

            gcn_layer(lambda hh: x_d[hh * p.half:
                                     min((hh + 1) * p.half, p.n_nodes), :],
                      w1_s, b1_s, H1D, sink1)

            # ---- exchange h1 shards
            nc.gpsimd.collective_compute(
                "AllGather", mybir.AluOpType.bypass,
                replica_groups=[list(range(p.n_cores))],
                ins=[h1_shard[:, :]], outs=[h1_full[:, :]])

            # ---- layer 2: h1_full -> h2T (kept on-chip, feat-major)
            def sink2(ww, wsz, ph, b_s):
                nc.scalar.activation(
                    h2T[:, ww * p.win: ww * p.win + wsz],
                    ph[:, :wsz], AF.Tanh, bias=b_s[:, 0:1])

            gcn_layer(lambda hh: h1_full[hh * p.half:
                                         min((hh + 1) * p.half, p.n_nodes), :],
                      w2_s, b2_s, H2D, sink2)

            # ---- fc layers on the dst shard
            for c0 in range(0, p.npc, NCHUNK):
                cs = min(NCHUNK, p.npc - c0)
                p3 = pfpool.tile([H3D, NCHUNK], f32, tag="p3")
                nc.tensor.matmul(p3[:, :cs], lhsT=w3_s[:],
                                 rhs=h2T[:, c0:c0 + cs],
                                 start=True, stop=True)
                h3 = wpool.tile([H3D, NCHUNK], f32, tag="h3")
                nc.scalar.activation(h3[:, :cs], p3[:, :cs], AF.Tanh,
                                     bias=b3_s[:, 0:1])
                p4 = pfpool.tile([OUTD, NCHUNK], f32, tag="p4")
                nc.tensor.matmul(p4[:, :cs], lhsT=w4_s[:], rhs=h3[:, :cs],
                                 start=True, stop=True)
                ob = wpool.tile([OUTD, NCHUNK], f32, tag="ob")
                nc.vector.tensor_scalar(
                    out=ob[:, :cs], in0=p4[:, :cs],
                    scalar1=b4_s[0:1, 0:1], scalar2=None, op0=OP.add)
                nc.sync.dma_start(out_d[c0:c0 + cs, :], ob[0:1, :cs])

    nc.compile()
    return nc


def make_in_maps(p, inputs):
    x = np.ascontiguousarray(np.asarray(inputs["x"], dtype=np.float32))
    iota = np.tile(np.arange(p.win, dtype=np.float32)[None, :], (128, 1))
    maps = []
    for c in range(p.n_cores):
        maps.append({
            "x": x,
            "midx": p.idx_arr[c],
            "mdloc": p.dloc_arr[c],
            "mcoef": p.coef_arr[c],
            "iota": iota,
            "w1": np.asarray(inputs["W1"], dtype=np.float32),
            "b1": np.asarray(inputs["b1"], dtype=np.float32).reshape(-1, 1),
            "w2": np.asarray(inputs["W2"], dtype=np.float32),
            "b2": np.asarray(inputs["b2"], dtype=np.float32).reshape(-1, 1),
            "w3": np.asarray(inputs["W3"], dtype=np.float32),
            "b3": np.asarray(inputs["b3"], dtype=np.float32).reshape(-1, 1),
            "w4": np.asarray(inputs["W4"], dtype=np.float32),
            "b4": np.asarray(inputs["b4"], dtype=np.float32).reshape(-1, 1),
        })
    return maps


_CACHE = {}


def kernel(_trace=False, **inputs):
    from concourse.bass_utils import run_bass_kernel_spmd

    edge_index = np.asarray(inputs["edge_index"])
    p = make_plan(edge_index)
    key = (p.S, tuple(int(c[3]) for c in p.calls))
    if key not in _CACHE:
        _CACHE[key] = build_program(p)
    nc = _CACHE[key]
    res = run_bass_kernel_spmd(nc, make_in_maps(p, inputs),
                               core_ids=list(range(p.n_cores)),
                               trace=_trace)
    out = np.concatenate([res.results[c]["out"] for c in range(p.n_cores)],
                         axis=0)
    if _trace:
        return out, res
    return out


# revision 14
# speedup vs baseline: 1.4482x; 1.4055x over previous
"""BrainGCN Trainium2 kernel (8 NeuronCores, Bass/Tile).

Model (PyG-style GCNConv x2 + 2 FC layers):
    h = tanh(gcn(x,  W1, b1)); h = tanh(gcn(h, W2, b2))
    h = tanh(h @ W3 + b3);      out = h @ W4 + b4

gcn(x, W, b) = (agg + x * dinv^2) @ W + b  with
    agg[d] = sum_{e:(s,d)} dinv[s]*dinv[d] * x[s]        (by linearity we
aggregate raw feature rows first, then apply W once per node).

Distribution: dst-nodes are split into 8 contiguous blocks (one per core).
Each core aggregates its own dst block; self-loops are folded in as ordinary
edges with coef = dinv^2.  The only cross-core exchange is an AllGather of
the h1 shards between the two GCN layers.

Device-side scatter-add: edges are sorted by dst into 128-wide dst windows.
For every 128-edge tile the DVE builds S[e, d_local] = coef[e] *
onehot(d_local[e]) with a single tensor_scalar (iota == dloc) * coef, and the
PE accumulates aggT[feat, dst] += E_tile^T @ S into PSUM.  Gathers of the
512-byte feature rows run on dma_gather (int16 indices -> the table is
addressed in halves of 25000 rows).
"""

import math

import numpy as np

# ---------------------------------------------------------------- constants
N_NODES = 50000
N_CORES = 8
F_IN, H1D, H2D, H3D, OUTD = 128, 128, 64, 64, 1
WIN = 128          # dst window width (psum free dim of the scatter matmul)
HALF = 25000       # gather-table half size (int16 index range)
G_WINDOWS = 6      # dst windows per gather chunk (per half)
NCHUNK = 512       # fc-layer column chunk


def _cdiv(a, b):
    return -(-a // b)


def _rup(a, b):
    return _cdiv(a, b) * b


# ------------------------------------------------------------------ planning
class Plan:
    pass


def make_plan(edge_index, n_nodes=N_NODES, n_cores=N_CORES, half=HALF,
              g_windows=G_WINDOWS, win=WIN):
    """Host-side graph preprocessing -> static schedule + per-core arrays."""
    src = np.asarray(edge_index[0]).astype(np.int64)
    dst = np.asarray(edge_index[1]).astype(np.int64)

    npc = n_nodes // n_cores
    assert npc * n_cores == n_nodes
    n_win = _cdiv(npc, win)
    n_half = _cdiv(n_nodes, half)
    assert half <= 32767

    deg = np.bincount(dst, minlength=n_nodes).astype(np.float64) + 1.0
    dinv = 1.0 / np.sqrt(deg)

    # append self loops as ordinary edges
    loops = np.arange(n_nodes, dtype=np.int64)
    s_all = np.concatenate([src, loops])
    d_all = np.concatenate([dst, loops])
    coef = (dinv[s_all] * dinv[d_all]).astype(np.float32)

    core = d_all // npc
    w = (d_all % npc) // win
    h = s_all // half

    # per (core, w, h) counts -> static caps shared by all cores
    gid = (core * n_win + w) * n_half + h
    counts = np.bincount(gid, minlength=n_cores * n_win * n_half)
    counts = counts.reshape(n_cores, n_win, n_half)
    caps = _rup(counts.max(axis=0), 128)          # [n_win, n_half] slots

    # window groups (gather chunks)
    wgroups = [list(range(i, min(i + g_windows, n_win)))
               for i in range(0, n_win, g_windows)]

    # static slot offsets, in [wg][h][w] order
    off = {}
    calls = []        # (wg_index, h, slot_off, n_slots)
    pos = 0
    for gi, wg in enumerate(wgroups):
        for hh in range(n_half):
            call_off = pos
            for ww in wg:
                off[(ww, hh)] = pos
                pos += int(caps[ww, hh])
            calls.append((gi, hh, call_off, pos - call_off))
    S = pos                                        # total slots (mult of 128)
    assert S % 128 == 0
    T = S // 128                                   # total tiles

    # per-window tile lists: [(h, tile_local_in_call, tile_global)]
    win_tiles = []
    for ww in range(n_win):
        tiles = []
        for hh in range(n_half):
            gi = ww // g_windows
            call_off = next(c[2] for c in calls if c[0] == gi and c[1] == hh)
            o = off[(ww, hh)]
            for t in range(int(caps[ww, hh]) // 128):
                tiles.append((hh, (o - call_off) // 128 + t, o // 128 + t))
        win_tiles.append(tiles)

    # ------- per-core arrays
    # order edges: core, then (wg, h, w) -- same as the slot layout
    wg_of_w = np.array([ww // g_windows for ww in range(n_win)])
    order_key = (((core * len(wgroups) + wg_of_w[w]) * n_half + h) * n_win + w)
    order = np.argsort(order_key, kind="stable")
    s_o, d_o, c_o = s_all[order], d_all[order], coef[order]
    core_o, w_o, h_o = core[order], w[order], h[order]

    # destination slot for each edge: static group offset + rank within group.
    # each (core, w, h) group is contiguous in the sorted order; rank = index
    # since the group's first element.
    comb = (core_o * n_win + w_o) * n_half + h_o
    pos = np.arange(comb.size, dtype=np.int64)
    is_start = np.ones(comb.size, dtype=bool)
    if comb.size > 1:
        is_start[1:] = comb[1:] != comb[:-1]
    rank = pos - np.maximum.accumulate(np.where(is_start, pos, 0))

    static_off = np.zeros((n_win, n_half), dtype=np.int64)
    for ww in range(n_win):
        for hh in range(n_half):
            static_off[ww, hh] = off[(ww, hh)]
    slot = static_off[w_o, h_o] + rank

    idx16 = np.zeros((n_cores, S), dtype=np.int16)
    dloc = np.zeros((n_cores, S), dtype=np.float32)
    cof = np.zeros((n_cores, S), dtype=np.float32)
    ci = core_o.astype(np.int64)
    idx16[ci, slot] = (s_o - h_o * half).astype(np.int16)
    dloc[ci, slot] = (d_o - ci * npc - w_o * win).astype(np.float32)
    cof[ci, slot] = c_o

    p = Plan()
    p.n_nodes, p.n_cores, p.npc = n_nodes, n_cores, npc
    p.win, p.n_win, p.half, p.n_half = win, n_win, half, n_half
    p.win_sizes = [min(win, npc - ww * win) for ww in range(n_win)]
    p.wgroups, p.calls, p.win_tiles = wgroups, calls, win_tiles
    p.S, p.T = S, T
    # layouts for the device: idx [128, S/16] (16-row wrap, replicated x8),
    # dloc/coef [128, T] (slot i -> [i%128, i//128])
    p.idx_arr = np.ascontiguousarray(
        np.tile(idx16.reshape(n_cores, S // 16, 16).transpose(0, 2, 1),
                (1, 8, 1)))
    p.dloc_arr = np.ascontiguousarray(dloc.reshape(n_cores, T, 128)
                                      .transpose(0, 2, 1))
    p.coef_arr = np.ascontiguousarray(cof.reshape(n_cores, T, 128)
                                      .transpose(0, 2, 1))
    return p


# ------------------------------------------------------------------- program
def build_program(p, debug=False, n_queues=4, scratch=32768, f32r=False,
                  edge_dt="float16"):
    import concourse.bacc as bacc
    import concourse.bass as bass
    import concourse.mybir as mybir
    import concourse.tile as tile
    from concourse.masks import make_identity

    f32 = mybir.dt.float32
    f32r = mybir.dt.float32r if f32r else f32
    edt = getattr(mybir.dt, edge_dt)
    i16 = mybir.dt.int16
    AF = mybir.ActivationFunctionType
    OP = mybir.AluOpType

    nc = bacc.Bacc("TRN2", target_bir_lowering=False, debug=debug,
                   num_devices=p.n_cores, num_swdge_queues=n_queues,
                   dynamic_dma_scratch_size=scratch)

    x_d = nc.dram_tensor("x", [p.n_nodes, F_IN], edt, kind="ExternalInput")
    idx_d = nc.dram_tensor("midx", [128, p.S // 16], i16, kind="ExternalInput")
    dloc_d = nc.dram_tensor("mdloc", [128, p.T], f32, kind="ExternalInput")
    coef_d = nc.dram_tensor("mcoef", [128, p.T], f32, kind="ExternalInput")
    iota_d = nc.dram_tensor("iota", [128, p.win], edt, kind="ExternalInput")
    w1_d = nc.dram_tensor("w1", [F_IN, H1D], f32, kind="ExternalInput")
    b1_d = nc.dram_tensor("b1", [H1D, 1], f32, kind="ExternalInput")
    w2_d = nc.dram_tensor("w2", [H1D, H2D], f32, kind="ExternalInput")
    b2_d = nc.dram_tensor("b2", [H2D, 1], f32, kind="ExternalInput")
    w3_d = nc.dram_tensor("w3", [H2D, H3D], f32, kind="ExternalInput")
    b3_d = nc.dram_tensor("b3", [H3D, 1], f32, kind="ExternalInput")
    w4_d = nc.dram_tensor("w4", [H3D, OUTD], f32, kind="ExternalInput")
    b4_d = nc.dram_tensor("b4", [OUTD, 1], f32, kind="ExternalInput")
    out_d = nc.dram_tensor("out", [p.npc, OUTD], f32, kind="ExternalOutput")

    h1_shard = nc.dram_tensor("h1_shard", [p.npc, H1D], edt)
    h1_full = nc.dram_tensor("h1_full", [p.n_nodes, H1D], edt,
                             addr_space="Shared")

    with tile.TileContext(nc) as tc:
        with (
            tc.tile_pool(name="const", bufs=1) as cpool,
            tc.tile_pool(name="gather", bufs=2) as gpool,
            tc.tile_pool(name="sel", bufs=4) as spool,
            tc.tile_pool(name="work", bufs=3) as wpool,
            tc.tile_pool(name="persist", bufs=1) as ppool,
            tc.tile_pool(name="psum", bufs=2, space="PSUM") as pspool,
            tc.tile_pool(name="psumfc", bufs=1, space="PSUM") as pfpool,
        ):
            # ---- constants / metadata to SBUF
            idx_s = cpool.tile([128, p.S // 16], i16)
            nc.sync.dma_start(idx_s[:], idx_d[:, :])
            dloc_s = cpool.tile([128, p.T], f32)
            nc.sync.dma_start(dloc_s[:], dloc_d[:, :])
            coef_s = cpool.tile([128, p.T], f32)
            nc.sync.dma_start(coef_s[:], coef_d[:, :])
            iota_s = cpool.tile([128, p.win], edt)
            nc.sync.dma_start(iota_s[:], iota_d[:, :])
            w1_s = cpool.tile([F_IN, H1D], f32)
            nc.sync.dma_start(w1_s[:], w1_d[:, :])
            b1_s = cpool.tile([H1D, 1], f32)
            nc.sync.dma_start(b1_s[:], b1_d[:, :])
            w2_s = cpool.tile([H1D, H2D], f32)
            nc.sync.dma_start(w2_s[:], w2_d[:, :])
            b2_s = cpool.tile([H2D, 1], f32)
            nc.sync.dma_start(b2_s[:], b2_d[:, :])
            w3_s = cpool.tile([H2D, H3D], f32)
            nc.sync.dma_start(w3_s[:], w3_d[:, :])
            b3_s = cpool.tile([H3D, 1], f32)
            nc.sync.dma_start(b3_s[:], b3_d[:, :])
            w4_s = cpool.tile([H3D, OUTD], f32)
            nc.sync.dma_start(w4_s[:], w4_d[:, :])
            b4_s = cpool.tile([OUTD, 1], f32)
            nc.sync.dma_start(b4_s[:], b4_d[:, :])
            ident = cpool.tile([128, 128], f32)
            make_identity(nc, ident[:])

            h2T = ppool.tile([H2D, p.npc], f32)

            # ---------------- one GCN layer ----------------
            call_seq = [0]

            def gcn_layer(table_ap_fn, w_s, b_s, out_feat, sink):
                for gi, wg in enumerate(p.wgroups):
                    bufs = {}
                    for (cgi, hh, call_off, n_call) in p.calls:
                        if cgi != gi or n_call == 0:
                            continue
                        gb = gpool.tile([128, (n_call // 128) * F_IN], edt,
                                        tag=f"gb{hh}")
                        out3d = gb[:].rearrange("q (t e) -> q t e", e=F_IN)
                        nc.gpsimd.dma_gather(
                            out_ap=out3d,
                            in_ap=table_ap_fn(hh),
                            idxs_ap=idx_s[:, call_off // 16:
                                          (call_off + n_call) // 16],
                            num_idxs=n_call,
                            num_idxs_reg=n_call,
                            elem_size=F_IN,
                            single_packet=False,
                            queue_num=call_seq[0] % n_queues,
                        )
                        call_seq[0] += 1
                        bufs[hh] = gb
                    for ww in wg:
                        tiles = p.win_tiles[ww]
                        if not tiles:
                            continue
                        wsz = p.win_sizes[ww]
                        pag = pspool.tile([128, p.win], f32, tag="pag")
                        for k, (hh, lt, gt) in enumerate(tiles):
                            st = spool.tile([128, p.win], edt, tag="st")
                            nc.vector.tensor_scalar(
                                out=st[:], in0=iota_s[:],
                                scalar1=dloc_s[:, gt:gt + 1],
                                scalar2=coef_s[:, gt:gt + 1],
                                op0=OP.is_equal, op1=OP.mult)
                            nc.tensor.matmul(
                                pag[:],
                                lhsT=bufs[hh][:, lt * F_IN:(lt + 1) * F_IN],
                                rhs=st[:],
                                start=(k == 0), stop=(k == len(tiles) - 1))
                        aggT = wpool.tile([128, p.win], f32, tag="aggT")
                        nc.vector.tensor_copy(aggT[:], pag[:])
                        ph = pspool.tile([out_feat, p.win], f32, tag="ph")
                        nc.tensor.matmul(ph[:], lhsT=w_s[:], rhs=aggT[:],
                                         start=True, stop=True)
                        sink(ww, wsz, ph, b_s)

            # ---- layer 1: x -> h1_shard (node-major, via PE transpose)
            def sink1(ww, wsz, ph, b_s):
                hT = wpool.tile([128, p.win], f32, tag="hT")
                nc.scalar.activation(hT[:], ph[:], AF.Tanh,
                                     bias=b_s[:, 0:1])
                pt = pspool.tile([128, 128], f32, tag="pt")
                nc.tensor.transpose(pt[:], hT[:], ident[:])
                hw_ = wpool.tile([128, 128], edt, tag="hw")
                nc.vector.tensor_copy(hw_[:], pt[:])
                nc.sync.dma_start(
                    h1_shard[ww * p.win: ww * p.win + wsz, :],
                    hw_[:wsz, :])

            gcn_layer(lambda hh: x_d[hh * p.half:
                                     min((hh + 1) * p.half, p.n_nodes), :],
                      w1_s, b1_s, H1D, sink1)

            # ---- exchange h1 shards
            nc.gpsimd.collective_compute(
                "AllGather", mybir.AluOpType.bypass,
                replica_groups=[list(range(p.n_cores))],
                ins=[h1_shard[:, :]], outs=[h1_full[:, :]])

            # ---- layer 2: h1_full -> h2T (kept on-chip, feat-major)
            def sink2(ww, wsz, ph, b_s):
                nc.scalar.activation(
                    h2T[:, ww * p.win: ww * p.win + wsz],
                    ph[:, :wsz], AF.Tanh, bias=b_s[:, 0:1])

            gcn_layer(lambda hh: h1_full[hh * p.half:
                                         min((hh + 1) * p.half, p.n_nodes), :],
                      w2_s, b2_s, H2D, sink2)

            # ---- fc layers on the dst shard
            for c0 in range(0, p.npc, NCHUNK):
                cs = min(NCHUNK, p.npc - c0)
                p3 = pfpool.tile([H3D, NCHUNK], f32, tag="p3")
                nc.tensor.matmul(p3[:, :cs], lhsT=w3_s[:],
                                 rhs=h2T[:, c0:c0 + cs],
                                 start=True, stop=True)
                h3 = wpool.tile([H3D, NCHUNK], f32, tag="h3")
                nc.scalar.activation(h3[:, :cs], p3[:, :cs], AF.Tanh,
                                     bias=b3_s[:, 0:1])
                p4 = pfpool.tile([OUTD, NCHUNK], f32, tag="p4")
                nc.tensor.matmul(p4[:, :cs], lhsT=w4_s[:], rhs=h3[:, :cs],
                                 start=True, stop=True)
                ob = wpool.tile([OUTD, NCHUNK], f32, tag="ob")
                nc.vector.tensor_scalar(
                    out=ob[:, :cs], in0=p4[:, :cs],
                    scalar1=b4_s[0:1, 0:1], scalar2=None, op0=OP.add)
                nc.sync.dma_start(out_d[c0:c0 + cs, :], ob[0:1, :cs])

    nc.compile()
    return nc


def make_in_maps(p, inputs, edge_dt="float16"):
    np_edt = dict(float32=np.float32, float16=np.float16)[edge_dt]
    if edge_dt == "bfloat16":
        import ml_dtypes
        np_edt = ml_dtypes.bfloat16
    x = np.ascontiguousarray(np.asarray(inputs["x"]).astype(np_edt))
    iota = np.tile(np.arange(p.win, dtype=np_edt)[None, :], (128, 1))
    maps = []
    for c in range(p.n_cores):
        maps.append({
            "x": x,
            "midx": p.idx_arr[c],
            "mdloc": p.dloc_arr[c],
            "mcoef": p.coef_arr[c],
            "iota": iota,
            "w1": np.asarray(inputs["W1"], dtype=np.float32),
            "b1": np.asarray(inputs["b1"], dtype=np.float32).reshape(-1, 1),
            "w2": np.asarray(inputs["W2"], dtype=np.float32),
            "b2": np.asarray(inputs["b2"], dtype=np.float32).reshape(-1, 1),
            "w3": np.asarray(inputs["W3"], dtype=np.float32),
            "b3": np.asarray(inputs["b3"], dtype=np.float32).reshape(-1, 1),
            "w4": np.asarray(inputs["W4"], dtype=np.float32),
            "b4": np.asarray(inputs["b4"], dtype=np.float32).reshape(-1, 1),
        })
    return maps


_CACHE = {}


def kernel(_trace=False, **inputs):
    from concourse.bass_utils import run_bass_kernel_spmd

    edge_index = np.asarray(inputs["edge_index"])
    p = make_plan(edge_index)
    key = (p.S, tuple(int(c[3]) for c in p.calls))
    if key not in _CACHE:
        _CACHE[key] = build_program(p)
    nc = _CACHE[key]
    res = run_bass_kernel_spmd(nc, make_in_maps(p, inputs),
                               core_ids=list(range(p.n_cores)),
                               trace=_trace)
    out = np.concatenate([res.results[c]["out"] for c in range(p.n_cores)],
                         axis=0)
    if _trace:
        return out, res
    return out


# revision 18
# speedup vs baseline: 2.2669x; 1.5654x over previous
"""BrainGCN Trainium2 kernel (8 NeuronCores, Bass/Tile).

Model (PyG-style GCNConv x2 + 2 FC layers):
    h = tanh(gcn(x,  W1, b1)); h = tanh(gcn(h, W2, b2))
    h = tanh(h @ W3 + b3);      out = h @ W4 + b4

gcn(x, W, b) = (agg + x * dinv^2) @ W + b  with
    agg[d] = sum_{e:(s,d)} dinv[s]*dinv[d] * x[s]        (by linearity we
aggregate raw feature rows first, then apply W once per node).

Distribution: dst-nodes are split into 8 contiguous blocks (one per core).
Each core aggregates its own dst block; self-loops are folded in as ordinary
edges with coef = dinv^2.  The only cross-core exchange is an AllGather of
the h1 shards between the two GCN layers.

Device-side scatter-add: edges are sorted by dst into 128-wide dst windows.
For every 128-edge tile the DVE builds S[e, d_local] = coef[e] *
onehot(d_local[e]) with a single tensor_scalar (iota == dloc) * coef, and the
PE accumulates aggT[feat, dst] += E_tile^T @ S into PSUM.  Gathers of the
512-byte feature rows run on dma_gather (int16 indices -> the table is
addressed in halves of 25000 rows).
"""

import math

import numpy as np

# ---------------------------------------------------------------- constants
N_NODES = 50000
N_CORES = 8
F_IN, H1D, H2D, H3D, OUTD = 128, 128, 64, 64, 1
WIN = 128          # dst window width (psum free dim of the scatter matmul)
HALF = 25000       # gather-table half size (int16 index range)
G_WINDOWS = 6      # dst windows per gather chunk (per half)
NCHUNK = 512       # fc-layer column chunk


def _cdiv(a, b):
    return -(-a // b)


def _rup(a, b):
    return _cdiv(a, b) * b


# ------------------------------------------------------------------ planning
class Plan:
    pass


def make_plan(edge_index, n_nodes=N_NODES, n_cores=N_CORES, half=HALF,
              g_windows=G_WINDOWS, win=WIN):
    """Host-side graph preprocessing -> static schedule + per-core arrays."""
    src = np.asarray(edge_index[0]).astype(np.int64)
    dst = np.asarray(edge_index[1]).astype(np.int64)

    npc = n_nodes // n_cores
    assert npc * n_cores == n_nodes
    n_win = _cdiv(npc, win)
    n_half = _cdiv(n_nodes, half)
    assert half <= 32767

    deg = np.bincount(dst, minlength=n_nodes).astype(np.float64) + 1.0
    dinv = 1.0 / np.sqrt(deg)

    # self-loops are handled separately (diag matmul); edges only here
    s_all, d_all = src, dst
    coef = (dinv[s_all] * dinv[d_all]).astype(np.float32)

    core = d_all // npc
    w = (d_all % npc) // win
    h = s_all // half

    # per (core, w, h) counts -> static caps shared by all cores
    gid = (core * n_win + w) * n_half + h
    counts = np.bincount(gid, minlength=n_cores * n_win * n_half)
    counts = counts.reshape(n_cores, n_win, n_half)
    caps = _rup(counts.max(axis=0), 128)          # [n_win, n_half] slots

    # window groups (gather chunks)
    wgroups = [list(range(i, min(i + g_windows, n_win)))
               for i in range(0, n_win, g_windows)]

    # static slot offsets, in [wg][h][w] order
    off = {}
    calls = []        # (wg_index, h, slot_off, n_slots)
    pos = 0
    for gi, wg in enumerate(wgroups):
        for hh in range(n_half):
            call_off = pos
            for ww in wg:
                off[(ww, hh)] = pos
                pos += int(caps[ww, hh])
            calls.append((gi, hh, call_off, pos - call_off))
    S = pos                                        # total slots (mult of 128)
    assert S % 128 == 0
    T = S // 128                                   # total tiles

    # per-window tile lists: [(h, tile_local_in_call, tile_global)]
    win_tiles = []
    for ww in range(n_win):
        tiles = []
        for hh in range(n_half):
            gi = ww // g_windows
            call_off = next(c[2] for c in calls if c[0] == gi and c[1] == hh)
            o = off[(ww, hh)]
            for t in range(int(caps[ww, hh]) // 128):
                tiles.append((hh, (o - call_off) // 128 + t, o // 128 + t))
        win_tiles.append(tiles)

    # ------- per-core arrays
    # order edges: core, then (wg, h, w) -- same as the slot layout
    wg_of_w = np.array([ww // g_windows for ww in range(n_win)])
    order_key = (((core * len(wgroups) + wg_of_w[w]) * n_half + h) * n_win + w)
    order = np.argsort(order_key, kind="stable")
    s_o, d_o, c_o = s_all[order], d_all[order], coef[order]
    core_o, w_o, h_o = core[order], w[order], h[order]

    # destination slot for each edge: static group offset + rank within group.
    # each (core, w, h) group is contiguous in the sorted order; rank = index
    # since the group's first element.
    comb = (core_o * n_win + w_o) * n_half + h_o
    pos = np.arange(comb.size, dtype=np.int64)
    is_start = np.ones(comb.size, dtype=bool)
    if comb.size > 1:
        is_start[1:] = comb[1:] != comb[:-1]
    rank = pos - np.maximum.accumulate(np.where(is_start, pos, 0))

    static_off = np.zeros((n_win, n_half), dtype=np.int64)
    for ww in range(n_win):
        for hh in range(n_half):
            static_off[ww, hh] = off[(ww, hh)]
    slot = static_off[w_o, h_o] + rank

    idx16 = np.zeros((n_cores, S), dtype=np.int16)
    dloc = np.zeros((n_cores, S), dtype=np.float32)
    cof = np.zeros((n_cores, S), dtype=np.float32)
    ci = core_o.astype(np.int64)
    idx16[ci, slot] = (s_o - h_o * half).astype(np.int16)
    dloc[ci, slot] = (d_o - ci * npc - w_o * win).astype(np.float32)
    cof[ci, slot] = c_o

    p = Plan()
    p.n_nodes, p.n_cores, p.npc = n_nodes, n_cores, npc
    p.win, p.n_win, p.half, p.n_half = win, n_win, half, n_half
    p.win_sizes = [min(win, npc - ww * win) for ww in range(n_win)]
    p.wgroups, p.calls, p.win_tiles = wgroups, calls, win_tiles
    p.S, p.T = S, T
    # layouts for the device: idx [128, S/16] (16-row wrap, replicated x8),
    # dloc/coef [128, T] (slot i -> [i%128, i//128])
    p.idx_arr = np.ascontiguousarray(
        np.tile(idx16.reshape(n_cores, S // 16, 16).transpose(0, 2, 1),
                (1, 8, 1)))

    # host-built S tiles: smat[c, p, t*win + j] = coef of (slot t*128+p -> j)
    smat = np.zeros((n_cores, S, win), dtype=np.float16)
    smat[ci, slot, dloc[ci, slot].astype(np.int64)] = cof[ci, slot]
    # zero out the untouched pad slots' spurious [0]-column hits:
    # pad slots have cof 0 so their writes are 0.0 anyway.
    p.smat = np.ascontiguousarray(
        smat.reshape(n_cores, T, 128, win).transpose(0, 2, 1, 3)
        .reshape(n_cores, 128, T * win))

    # self-loop diagonal: sdiag[c, p, w*win + j] = (p==j)*dinv^2[global node]
    d2 = (dinv * dinv).astype(np.float32)
    sdiag = np.zeros((n_cores, 128, n_win * win), dtype=np.float16)
    for c in range(n_cores):
        for ww in range(n_win):
            wsz = min(win, npc - ww * win)
            g0 = c * npc + ww * win
            sdiag[c, np.arange(wsz), ww * win + np.arange(wsz)] = d2[g0:g0 + wsz]
    p.sdiag = sdiag

    # per-core tile range of each window group (for S streaming)
    wg_tiles = []
    pos2 = 0
    for gi, wg in enumerate(wgroups):
        n = sum(int(caps[ww, hh]) for hh in range(n_half) for ww in wg) // 128
        wg_tiles.append((pos2, n))
        pos2 += n
    p.wg_tiles = wg_tiles
    return p


# ------------------------------------------------------------------- program
def build_program(p, debug=False, n_queues=4, scratch=32768, f32r=False,
                  edge_dt="float16"):
    import concourse.bacc as bacc
    import concourse.bass as bass
    import concourse.mybir as mybir
    import concourse.tile as tile
    from concourse.masks import make_identity

    f32 = mybir.dt.float32
    f32r = mybir.dt.float32r if f32r else f32
    edt = getattr(mybir.dt, edge_dt)
    i16 = mybir.dt.int16
    AF = mybir.ActivationFunctionType
    OP = mybir.AluOpType

    nc = bacc.Bacc("TRN2", target_bir_lowering=False, debug=debug,
                   num_devices=p.n_cores, num_swdge_queues=n_queues,
                   dynamic_dma_scratch_size=scratch)

    x_d = nc.dram_tensor("x", [p.n_nodes, F_IN], edt, kind="ExternalInput")
    idx_d = nc.dram_tensor("midx", [128, p.S // 16], i16, kind="ExternalInput")
    smat_d = nc.dram_tensor("smat", [128, p.T * p.win], edt,
                            kind="ExternalInput")
    sdiag_d = nc.dram_tensor("sdiag", [128, p.n_win * p.win], edt,
                             kind="ExternalInput")
    xown_d = nc.dram_tensor("xown", [p.npc, F_IN], edt, kind="ExternalInput")
    w1_d = nc.dram_tensor("w1", [F_IN, H1D], f32, kind="ExternalInput")
    b1_d = nc.dram_tensor("b1", [H1D, 1], f32, kind="ExternalInput")
    w2_d = nc.dram_tensor("w2", [H1D, H2D], f32, kind="ExternalInput")
    b2_d = nc.dram_tensor("b2", [H2D, 1], f32, kind="ExternalInput")
    w3_d = nc.dram_tensor("w3", [H2D, H3D], f32, kind="ExternalInput")
    b3_d = nc.dram_tensor("b3", [H3D, 1], f32, kind="ExternalInput")
    w4_d = nc.dram_tensor("w4", [H3D, OUTD], f32, kind="ExternalInput")
    b4_d = nc.dram_tensor("b4", [OUTD, 1], f32, kind="ExternalInput")
    out_d = nc.dram_tensor("out", [p.npc, OUTD], f32, kind="ExternalOutput")

    h1_shard = nc.dram_tensor("h1_shard", [p.npc, H1D], edt)
    h1_full = nc.dram_tensor("h1_full", [p.n_nodes, H1D], edt,
                             addr_space="Shared")

    with tile.TileContext(nc) as tc:
        with (
            tc.tile_pool(name="const", bufs=1) as cpool,
            tc.tile_pool(name="gather", bufs=2) as gpool,
            tc.tile_pool(name="sel", bufs=2) as spool,
            tc.tile_pool(name="work", bufs=3) as wpool,
            tc.tile_pool(name="persist", bufs=1) as ppool,
            tc.tile_pool(name="psum", bufs=2, space="PSUM") as pspool,
            tc.tile_pool(name="psumfc", bufs=1, space="PSUM") as pfpool,
        ):
            # ---- constants / metadata to SBUF
            idx_s = cpool.tile([128, p.S // 16], i16)
            nc.sync.dma_start(idx_s[:], idx_d[:, :])
            sdiag_s = cpool.tile([128, p.n_win * p.win], edt)
            nc.sync.dma_start(sdiag_s[:], sdiag_d[:, :])
            w1_s = cpool.tile([F_IN, H1D], f32)
            nc.sync.dma_start(w1_s[:], w1_d[:, :])
            b1_s = cpool.tile([H1D, 1], f32)
            nc.sync.dma_start(b1_s[:], b1_d[:, :])
            w2_s = cpool.tile([H1D, H2D], f32)
            nc.sync.dma_start(w2_s[:], w2_d[:, :])
            b2_s = cpool.tile([H2D, 1], f32)
            nc.sync.dma_start(b2_s[:], b2_d[:, :])
            w3_s = cpool.tile([H2D, H3D], f32)
            nc.sync.dma_start(w3_s[:], w3_d[:, :])
            b3_s = cpool.tile([H3D, 1], f32)
            nc.sync.dma_start(b3_s[:], b3_d[:, :])
            w4_s = cpool.tile([H3D, OUTD], f32)
            nc.sync.dma_start(w4_s[:], w4_d[:, :])
            b4_s = cpool.tile([OUTD, 1], f32)
            nc.sync.dma_start(b4_s[:], b4_d[:, :])
            ident = cpool.tile([128, 128], f32)
            make_identity(nc, ident[:])

            h2T = ppool.tile([H2D, p.npc], f32)

            # ---------------- one GCN layer ----------------
            call_seq = [0]

            def gcn_layer(table_ap_fn, own_ap, w_s, b_s, out_feat, sink):
                for gi, wg in enumerate(p.wgroups):
                    bufs = {}
                    for (cgi, hh, call_off, n_call) in p.calls:
                        if cgi != gi or n_call == 0:
                            continue
                        gb = gpool.tile([128, (n_call // 128) * F_IN], edt,
                                        tag=f"gb{hh}")
                        out3d = gb[:].rearrange("q (t e) -> q t e", e=F_IN)
                        nc.gpsimd.dma_gather(
                            out_ap=out3d,
                            in_ap=table_ap_fn(hh),
                            idxs_ap=idx_s[:, call_off // 16:
                                          (call_off + n_call) // 16],
                            num_idxs=n_call,
                            num_idxs_reg=n_call,
                            elem_size=F_IN,
                            single_packet=False,
                            queue_num=call_seq[0] % n_queues,
                        )
                        call_seq[0] += 1
                        bufs[hh] = gb
                    # stream this window group's host-built S tiles
                    t0g, ntg = p.wg_tiles[gi]
                    sbf = spool.tile([128, ntg * p.win], edt, tag="sbf")
                    nc.sync.dma_start(
                        sbf[:], smat_d[:, t0g * p.win:(t0g + ntg) * p.win])
                    for ww in wg:
                        tiles = p.win_tiles[ww]
                        wsz = p.win_sizes[ww]
                        pag = pspool.tile([128, p.win], f32, tag="pag")
                        # self-loop term: x_own[window]^T @ diag(dinv^2)
                        xw_own = wpool.tile([128, F_IN], edt, tag="xo")
                        nc.sync.dma_start(
                            xw_own[:wsz, :],
                            own_ap[ww * p.win: ww * p.win + wsz, :])
                        nc.tensor.matmul(
                            pag[:], lhsT=xw_own[:wsz, :],
                            rhs=sdiag_s[:wsz,
                                        ww * p.win: (ww + 1) * p.win],
                            start=True, stop=(not tiles))
                        for k, (hh, lt, gt) in enumerate(tiles):
                            nc.tensor.matmul(
                                pag[:],
                                lhsT=bufs[hh][:, lt * F_IN:(lt + 1) * F_IN],
                                rhs=sbf[:, (gt - t0g) * p.win:
                                        (gt - t0g + 1) * p.win],
                                start=False, stop=(k == len(tiles) - 1))
                        aggT = wpool.tile([128, p.win], f32, tag="aggT")
                        nc.vector.tensor_copy(aggT[:], pag[:])
                        ph = pspool.tile([out_feat, p.win], f32, tag="ph")
                        nc.tensor.matmul(ph[:], lhsT=w_s[:], rhs=aggT[:],
                                         start=True, stop=True)
                        sink(ww, wsz, ph, b_s)

            # ---- layer 1: x -> h1_shard (node-major, via PE transpose)
            def sink1(ww, wsz, ph, b_s):
                hT = wpool.tile([128, p.win], f32, tag="hT")
                nc.scalar.activation(hT[:], ph[:], AF.Tanh,
                                     bias=b_s[:, 0:1])
                pt = pspool.tile([128, 128], f32, tag="pt")
                nc.tensor.transpose(pt[:], hT[:], ident[:])
                hw_ = wpool.tile([128, 128], edt, tag="hw")
                nc.vector.tensor_copy(hw_[:], pt[:])
                nc.sync.dma_start(
                    h1_shard[ww * p.win: ww * p.win + wsz, :],
                    hw_[:wsz, :])

            gcn_layer(lambda hh: x_d[hh * p.half:
                                     min((hh + 1) * p.half, p.n_nodes), :],
                      xown_d, w1_s, b1_s, H1D, sink1)

            # ---- exchange h1 shards
            nc.gpsimd.collective_compute(
                "AllGather", mybir.AluOpType.bypass,
                replica_groups=[list(range(p.n_cores))],
                ins=[h1_shard[:, :]], outs=[h1_full[:, :]])

            # ---- layer 2: h1_full -> h2T (kept on-chip, feat-major)
            def sink2(ww, wsz, ph, b_s):
                nc.scalar.activation(
                    h2T[:, ww * p.win: ww * p.win + wsz],
                    ph[:, :wsz], AF.Tanh, bias=b_s[:, 0:1])

            gcn_layer(lambda hh: h1_full[hh * p.half:
                                         min((hh + 1) * p.half, p.n_nodes), :],
                      h1_shard, w2_s, b2_s, H2D, sink2)

            # ---- fc layers on the dst shard
            for c0 in range(0, p.npc, NCHUNK):
                cs = min(NCHUNK, p.npc - c0)
                p3 = pfpool.tile([H3D, NCHUNK], f32, tag="p3")
                nc.tensor.matmul(p3[:, :cs], lhsT=w3_s[:],
                                 rhs=h2T[:, c0:c0 + cs],
                                 start=True, stop=True)
                h3 = wpool.tile([H3D, NCHUNK], f32, tag="h3")
                nc.scalar.activation(h3[:, :cs], p3[:, :cs], AF.Tanh,
                                     bias=b3_s[:, 0:1])
                p4 = pfpool.tile([OUTD, NCHUNK], f32, tag="p4")
                nc.tensor.matmul(p4[:, :cs], lhsT=w4_s[:], rhs=h3[:, :cs],
                                 start=True, stop=True)
                ob = wpool.tile([OUTD, NCHUNK], f32, tag="ob")
                nc.vector.tensor_scalar(
                    out=ob[:, :cs], in0=p4[:, :cs],
                    scalar1=b4_s[0:1, 0:1], scalar2=None, op0=OP.add)
                nc.sync.dma_start(out_d[c0:c0 + cs, :], ob[0:1, :cs])

    nc.compile()
    return nc


def make_in_maps(p, inputs, edge_dt="float16"):
    np_edt = dict(float32=np.float32, float16=np.float16)[edge_dt]
    if edge_dt == "bfloat16":
        import ml_dtypes
        np_edt = ml_dtypes.bfloat16
    x = np.ascontiguousarray(np.asarray(inputs["x"]).astype(np_edt))
    maps = []
    for c in range(p.n_cores):
        maps.append({
            "x": x,
            "midx": p.idx_arr[c],
            "smat": p.smat[c].astype(np_edt, copy=False),
            "sdiag": p.sdiag[c].astype(np_edt, copy=False),
            "xown": x[c * p.npc:(c + 1) * p.npc],
            "w1": np.asarray(inputs["W1"], dtype=np.float32),
            "b1": np.asarray(inputs["b1"], dtype=np.float32).reshape(-1, 1),
            "w2": np.asarray(inputs["W2"], dtype=np.float32),
            "b2": np.asarray(inputs["b2"], dtype=np.float32).reshape(-1, 1),
            "w3": np.asarray(inputs["W3"], dtype=np.float32),
            "b3": np.asarray(inputs["b3"], dtype=np.float32).reshape(-1, 1),
            "w4": np.asarray(inputs["W4"], dtype=np.float32),
            "b4": np.asarray(inputs["b4"], dtype=np.float32).reshape(-1, 1),
        })
    return maps


_CACHE = {}


def kernel(_trace=False, **inputs):
    from concourse.bass_utils import run_bass_kernel_spmd

    edge_index = np.asarray(inputs["edge_index"])
    p = make_plan(edge_index)
    key = (p.S, tuple(int(c[3]) for c in p.calls))
    if key not in _CACHE:
        _CACHE[key] = build_program(p)
    nc = _CACHE[key]
    res = run_bass_kernel_spmd(nc, make_in_maps(p, inputs),
                               core_ids=list(range(p.n_cores)),
                               trace=_trace)
    out = np.concatenate([res.results[c]["out"] for c in range(p.n_cores)],
                         axis=0)
    if _trace:
        return out, res
    return out


# revision 19
# speedup vs baseline: 2.7136x; 1.1971x over previous
"""BrainGCN Trainium2 kernel (8 NeuronCores, Bass/Tile).

Model (PyG-style GCNConv x2 + 2 FC layers):
    h = tanh(gcn(x,  W1, b1)); h = tanh(gcn(h, W2, b2))
    h = tanh(h @ W3 + b3);      out = h @ W4 + b4

gcn(x, W, b) = (agg + x * dinv^2) @ W + b  with
    agg[d] = sum_{e:(s,d)} dinv[s]*dinv[d] * x[s]        (by linearity we
aggregate raw feature rows first, then apply W once per node).

Distribution: dst-nodes are split into 8 contiguous blocks (one per core).
Each core aggregates its own dst block; self-loops are folded in as ordinary
edges with coef = dinv^2.  The only cross-core exchange is an AllGather of
the h1 shards between the two GCN layers.

Device-side scatter-add: edges are sorted by dst into 128-wide dst windows.
For every 128-edge tile the DVE builds S[e, d_local] = coef[e] *
onehot(d_local[e]) with a single tensor_scalar (iota == dloc) * coef, and the
PE accumulates aggT[feat, dst] += E_tile^T @ S into PSUM.  Gathers of the
512-byte feature rows run on dma_gather (int16 indices -> the table is
addressed in halves of 25000 rows).
"""

import math

import numpy as np

# ---------------------------------------------------------------- constants
N_NODES = 50000
N_CORES = 8
F_IN, H1D, H2D, H3D, OUTD = 128, 128, 64, 64, 1
WIN = 128          # dst window width (psum free dim of the scatter matmul)
HALF = 25000       # gather-table half size (int16 index range)
G_WINDOWS = 3      # dst windows per gather chunk (per half)
NCHUNK = 512       # fc-layer column chunk


def _cdiv(a, b):
    return -(-a // b)


def _rup(a, b):
    return _cdiv(a, b) * b


# ------------------------------------------------------------------ planning
class Plan:
    pass


def make_plan(edge_index, n_nodes=N_NODES, n_cores=N_CORES, half=HALF,
              g_windows=G_WINDOWS, win=WIN):
    """Host-side graph preprocessing -> static schedule + per-core arrays."""
    src = np.asarray(edge_index[0]).astype(np.int64)
    dst = np.asarray(edge_index[1]).astype(np.int64)

    npc = n_nodes // n_cores
    assert npc * n_cores == n_nodes
    n_win = _cdiv(npc, win)
    n_half = _cdiv(n_nodes, half)
    assert half <= 32767

    deg = np.bincount(dst, minlength=n_nodes).astype(np.float64) + 1.0
    dinv = 1.0 / np.sqrt(deg)

    # self-loops are handled separately (diag matmul); edges only here
    s_all, d_all = src, dst
    coef = (dinv[s_all] * dinv[d_all]).astype(np.float32)

    core = d_all // npc
    w = (d_all % npc) // win
    h = s_all // half

    # per (core, w, h) counts -> static caps shared by all cores
    gid = (core * n_win + w) * n_half + h
    counts = np.bincount(gid, minlength=n_cores * n_win * n_half)
    counts = counts.reshape(n_cores, n_win, n_half)
    caps = _rup(counts.max(axis=0), 128)          # [n_win, n_half] slots

    # window groups (gather chunks)
    wgroups = [list(range(i, min(i + g_windows, n_win)))
               for i in range(0, n_win, g_windows)]

    # static slot offsets, in [wg][h][w] order
    off = {}
    calls = []        # (wg_index, h, slot_off, n_slots)
    pos = 0
    for gi, wg in enumerate(wgroups):
        for hh in range(n_half):
            call_off = pos
            for ww in wg:
                off[(ww, hh)] = pos
                pos += int(caps[ww, hh])
            calls.append((gi, hh, call_off, pos - call_off))
    S = pos                                        # total slots (mult of 128)
    assert S % 128 == 0
    T = S // 128                                   # total tiles

    # per-window tile lists: [(h, tile_local_in_call, tile_global)]
    win_tiles = []
    for ww in range(n_win):
        tiles = []
        for hh in range(n_half):
            gi = ww // g_windows
            call_off = next(c[2] for c in calls if c[0] == gi and c[1] == hh)
            o = off[(ww, hh)]
            for t in range(int(caps[ww, hh]) // 128):
                tiles.append((hh, (o - call_off) // 128 + t, o // 128 + t))
        win_tiles.append(tiles)

    # ------- per-core arrays
    # order edges: core, then (wg, h, w) -- same as the slot layout
    wg_of_w = np.array([ww // g_windows for ww in range(n_win)])
    order_key = (((core * len(wgroups) + wg_of_w[w]) * n_half + h) * n_win + w)
    order = np.argsort(order_key, kind="stable")
    s_o, d_o, c_o = s_all[order], d_all[order], coef[order]
    core_o, w_o, h_o = core[order], w[order], h[order]

    # destination slot for each edge: static group offset + rank within group.
    # each (core, w, h) group is contiguous in the sorted order; rank = index
    # since the group's first element.
    comb = (core_o * n_win + w_o) * n_half + h_o
    pos = np.arange(comb.size, dtype=np.int64)
    is_start = np.ones(comb.size, dtype=bool)
    if comb.size > 1:
        is_start[1:] = comb[1:] != comb[:-1]
    rank = pos - np.maximum.accumulate(np.where(is_start, pos, 0))

    static_off = np.zeros((n_win, n_half), dtype=np.int64)
    for ww in range(n_win):
        for hh in range(n_half):
            static_off[ww, hh] = off[(ww, hh)]
    slot = static_off[w_o, h_o] + rank

    idx16 = np.zeros((n_cores, S), dtype=np.int16)
    dloc = np.zeros((n_cores, S), dtype=np.float32)
    cof = np.zeros((n_cores, S), dtype=np.float32)
    ci = core_o.astype(np.int64)
    idx16[ci, slot] = (s_o - h_o * half).astype(np.int16)
    dloc[ci, slot] = (d_o - ci * npc - w_o * win).astype(np.float32)
    cof[ci, slot] = c_o

    p = Plan()
    p.n_nodes, p.n_cores, p.npc = n_nodes, n_cores, npc
    p.win, p.n_win, p.half, p.n_half = win, n_win, half, n_half
    p.win_sizes = [min(win, npc - ww * win) for ww in range(n_win)]
    p.wgroups, p.calls, p.win_tiles = wgroups, calls, win_tiles
    p.S, p.T = S, T
    # layouts for the device: idx [128, S/16] (16-row wrap, replicated x8),
    # dloc/coef [128, T] (slot i -> [i%128, i//128])
    p.idx_arr = np.ascontiguousarray(
        np.tile(idx16.reshape(n_cores, S // 16, 16).transpose(0, 2, 1),
                (1, 8, 1)))

    # host-built S tiles: smat[c, p, t*win + j] = coef of (slot t*128+p -> j)
    smat = np.zeros((n_cores, S, win), dtype=np.float16)
    smat[ci, slot, dloc[ci, slot].astype(np.int64)] = cof[ci, slot]
    # zero out the untouched pad slots' spurious [0]-column hits:
    # pad slots have cof 0 so their writes are 0.0 anyway.
    p.smat = np.ascontiguousarray(
        smat.reshape(n_cores, T, 128, win).transpose(0, 2, 1, 3)
        .reshape(n_cores, 128, T * win))

    # self-loop diagonal: sdiag[c, p, w*win + j] = (p==j)*dinv^2[global node]
    d2 = (dinv * dinv).astype(np.float32)
    sdiag = np.zeros((n_cores, 128, n_win * win), dtype=np.float16)
    for c in range(n_cores):
        for ww in range(n_win):
            wsz = min(win, npc - ww * win)
            g0 = c * npc + ww * win
            sdiag[c, np.arange(wsz), ww * win + np.arange(wsz)] = d2[g0:g0 + wsz]
    p.sdiag = sdiag

    # per-core tile range of each window group (for S streaming)
    wg_tiles = []
    pos2 = 0
    for gi, wg in enumerate(wgroups):
        n = sum(int(caps[ww, hh]) for hh in range(n_half) for ww in wg) // 128
        wg_tiles.append((pos2, n))
        pos2 += n
    p.wg_tiles = wg_tiles
    return p


# ------------------------------------------------------------------- program
def build_program(p, debug=False, n_queues=4, scratch=32768, f32r=False,
                  edge_dt="float16"):
    import concourse.bacc as bacc
    import concourse.bass as bass
    import concourse.mybir as mybir
    import concourse.tile as tile
    from concourse.masks import make_identity

    f32 = mybir.dt.float32
    f32r = mybir.dt.float32r if f32r else f32
    edt = getattr(mybir.dt, edge_dt)
    i16 = mybir.dt.int16
    AF = mybir.ActivationFunctionType
    OP = mybir.AluOpType

    nc = bacc.Bacc("TRN2", target_bir_lowering=False, debug=debug,
                   num_devices=p.n_cores, num_swdge_queues=n_queues,
                   dynamic_dma_scratch_size=scratch)

    x_d = nc.dram_tensor("x", [p.n_nodes, F_IN], edt, kind="ExternalInput")
    idx_d = nc.dram_tensor("midx", [128, p.S // 16], i16, kind="ExternalInput")
    smat_d = nc.dram_tensor("smat", [128, p.T * p.win], edt,
                            kind="ExternalInput")
    sdiag_d = nc.dram_tensor("sdiag", [128, p.n_win * p.win], edt,
                             kind="ExternalInput")
    xown_d = nc.dram_tensor("xown", [p.npc, F_IN], edt, kind="ExternalInput")
    w1_d = nc.dram_tensor("w1", [F_IN, H1D], f32, kind="ExternalInput")
    b1_d = nc.dram_tensor("b1", [H1D, 1], f32, kind="ExternalInput")
    w2_d = nc.dram_tensor("w2", [H1D, H2D], f32, kind="ExternalInput")
    b2_d = nc.dram_tensor("b2", [H2D, 1], f32, kind="ExternalInput")
    w3_d = nc.dram_tensor("w3", [H2D, H3D], f32, kind="ExternalInput")
    b3_d = nc.dram_tensor("b3", [H3D, 1], f32, kind="ExternalInput")
    w4_d = nc.dram_tensor("w4", [H3D, OUTD], f32, kind="ExternalInput")
    b4_d = nc.dram_tensor("b4", [OUTD, 1], f32, kind="ExternalInput")
    out_d = nc.dram_tensor("out", [p.npc, OUTD], f32, kind="ExternalOutput")

    h1_shard = nc.dram_tensor("h1_shard", [p.npc, H1D], edt)
    h1_full = nc.dram_tensor("h1_full", [p.n_nodes, H1D], edt,
                             addr_space="Shared")

    with tile.TileContext(nc) as tc:
        with (
            tc.tile_pool(name="const", bufs=1) as cpool,
            tc.tile_pool(name="gather", bufs=3) as gpool,
            tc.tile_pool(name="sel", bufs=3) as spool,
            tc.tile_pool(name="work", bufs=3) as wpool,
            tc.tile_pool(name="persist", bufs=1) as ppool,
            tc.tile_pool(name="psum", bufs=2, space="PSUM") as pspool,
            tc.tile_pool(name="psumfc", bufs=1, space="PSUM") as pfpool,
        ):
            # ---- constants / metadata to SBUF
            idx_s = cpool.tile([128, p.S // 16], i16)
            nc.sync.dma_start(idx_s[:], idx_d[:, :])
            sdiag_s = cpool.tile([128, p.n_win * p.win], edt)
            nc.sync.dma_start(sdiag_s[:], sdiag_d[:, :])
            w1_s = cpool.tile([F_IN, H1D], f32)
            nc.sync.dma_start(w1_s[:], w1_d[:, :])
            b1_s = cpool.tile([H1D, 1], f32)
            nc.sync.dma_start(b1_s[:], b1_d[:, :])
            w2_s = cpool.tile([H1D, H2D], f32)
            nc.sync.dma_start(w2_s[:], w2_d[:, :])
            b2_s = cpool.tile([H2D, 1], f32)
            nc.sync.dma_start(b2_s[:], b2_d[:, :])
            w3_s = cpool.tile([H2D, H3D], f32)
            nc.sync.dma_start(w3_s[:], w3_d[:, :])
            b3_s = cpool.tile([H3D, 1], f32)
            nc.sync.dma_start(b3_s[:], b3_d[:, :])
            w4_s = cpool.tile([H3D, OUTD], f32)
            nc.sync.dma_start(w4_s[:], w4_d[:, :])
            b4_s = cpool.tile([OUTD, 1], f32)
            nc.sync.dma_start(b4_s[:], b4_d[:, :])
            ident = cpool.tile([128, 128], f32)
            make_identity(nc, ident[:])

            h2T = ppool.tile([H2D, p.npc], f32)

            # ---------------- one GCN layer ----------------
            call_seq = [0]

            def gcn_layer(table_ap_fn, own_ap, w_s, b_s, out_feat, sink):
                for gi, wg in enumerate(p.wgroups):
                    bufs = {}
                    for (cgi, hh, call_off, n_call) in p.calls:
                        if cgi != gi or n_call == 0:
                            continue
                        gb = gpool.tile([128, (n_call // 128) * F_IN], edt,
                                        tag=f"gb{hh}")
                        out3d = gb[:].rearrange("q (t e) -> q t e", e=F_IN)
                        nc.gpsimd.dma_gather(
                            out_ap=out3d,
                            in_ap=table_ap_fn(hh),
                            idxs_ap=idx_s[:, call_off // 16:
                                          (call_off + n_call) // 16],
                            num_idxs=n_call,
                            num_idxs_reg=n_call,
                            elem_size=F_IN,
                            single_packet=False,
                            queue_num=call_seq[0] % n_queues,
                        )
                        call_seq[0] += 1
                        bufs[hh] = gb
                    # stream this window group's host-built S tiles
                    t0g, ntg = p.wg_tiles[gi]
                    sbf = spool.tile([128, ntg * p.win], edt, tag="sbf")
                    nc.sync.dma_start(
                        sbf[:], smat_d[:, t0g * p.win:(t0g + ntg) * p.win])
                    nw0 = wg[0] * p.win
                    nrows = min(p.npc, (wg[-1] + 1) * p.win) - nw0
                    xo = wpool.tile([128, len(wg) * F_IN], edt, tag="xo")
                    xo3 = xo[:].rearrange("q (t e) -> q t e", e=F_IN)
                    for j, ww in enumerate(wg):
                        wsz = p.win_sizes[ww]
                        nc.sync.dma_start(
                            xo3[:wsz, j, :],
                            own_ap[ww * p.win: ww * p.win + wsz, :])
                    for ww in wg:
                        tiles = p.win_tiles[ww]
                        wsz = p.win_sizes[ww]
                        j = ww - wg[0]
                        pag = pspool.tile([128, p.win], f32, tag="pag")
                        # self-loop term: x_own[window]^T @ diag(dinv^2)
                        nc.tensor.matmul(
                            pag[:], lhsT=xo[:wsz, j * F_IN:(j + 1) * F_IN],
                            rhs=sdiag_s[:wsz,
                                        ww * p.win: (ww + 1) * p.win],
                            start=True, stop=(not tiles))
                        for k, (hh, lt, gt) in enumerate(tiles):
                            nc.tensor.matmul(
                                pag[:],
                                lhsT=bufs[hh][:, lt * F_IN:(lt + 1) * F_IN],
                                rhs=sbf[:, (gt - t0g) * p.win:
                                        (gt - t0g + 1) * p.win],
                                start=False, stop=(k == len(tiles) - 1))
                        aggT = wpool.tile([128, p.win], f32, tag="aggT")
                        nc.vector.tensor_copy(aggT[:], pag[:])
                        ph = pspool.tile([out_feat, p.win], f32, tag="ph")
                        nc.tensor.matmul(ph[:], lhsT=w_s[:], rhs=aggT[:],
                                         start=True, stop=True)
                        sink(ww, wsz, ph, b_s)

            # ---- layer 1: x -> h1_shard (node-major, via PE transpose)
            def sink1(ww, wsz, ph, b_s):
                hT = wpool.tile([128, p.win], f32, tag="hT")
                nc.scalar.activation(hT[:], ph[:], AF.Tanh,
                                     bias=b_s[:, 0:1])
                pt = pspool.tile([128, 128], f32, tag="pt")
                nc.tensor.transpose(pt[:], hT[:], ident[:])
                hw_ = wpool.tile([128, 128], edt, tag="hw")
                nc.vector.tensor_copy(hw_[:], pt[:])
                nc.sync.dma_start(
                    h1_shard[ww * p.win: ww * p.win + wsz, :],
                    hw_[:wsz, :])

            gcn_layer(lambda hh: x_d[hh * p.half:
                                     min((hh + 1) * p.half, p.n_nodes), :],
                      xown_d, w1_s, b1_s, H1D, sink1)

            # ---- exchange h1 shards
            nc.gpsimd.collective_compute(
                "AllGather", mybir.AluOpType.bypass,
                replica_groups=[list(range(p.n_cores))],
                ins=[h1_shard[:, :]], outs=[h1_full[:, :]])

            # ---- layer 2: h1_full -> h2T (kept on-chip, feat-major)
            def sink2(ww, wsz, ph, b_s):
                nc.scalar.activation(
                    h2T[:, ww * p.win: ww * p.win + wsz],
                    ph[:, :wsz], AF.Tanh, bias=b_s[:, 0:1])

            gcn_layer(lambda hh: h1_full[hh * p.half:
                                         min((hh + 1) * p.half, p.n_nodes), :],
                      h1_shard, w2_s, b2_s, H2D, sink2)

            # ---- fc layers on the dst shard
            for c0 in range(0, p.npc, NCHUNK):
                cs = min(NCHUNK, p.npc - c0)
                p3 = pfpool.tile([H3D, NCHUNK], f32, tag="p3")
                nc.tensor.matmul(p3[:, :cs], lhsT=w3_s[:],
                                 rhs=h2T[:, c0:c0 + cs],
                                 start=True, stop=True)
                h3 = wpool.tile([H3D, NCHUNK], f32, tag="h3")
                nc.scalar.activation(h3[:, :cs], p3[:, :cs], AF.Tanh,
                                     bias=b3_s[:, 0:1])
                p4 = pfpool.tile([OUTD, NCHUNK], f32, tag="p4")
                nc.tensor.matmul(p4[:, :cs], lhsT=w4_s[:], rhs=h3[:, :cs],
                                 start=True, stop=True)
                ob = wpool.tile([OUTD, NCHUNK], f32, tag="ob")
                nc.vector.tensor_scalar(
                    out=ob[:, :cs], in0=p4[:, :cs],
                    scalar1=b4_s[0:1, 0:1], scalar2=None, op0=OP.add)
                nc.sync.dma_start(out_d[c0:c0 + cs, :], ob[0:1, :cs])

    nc.compile()
    return nc


def make_in_maps(p, inputs, edge_dt="float16"):
    np_edt = dict(float32=np.float32, float16=np.float16)[edge_dt]
    if edge_dt == "bfloat16":
        import ml_dtypes
        np_edt = ml_dtypes.bfloat16
    x = np.ascontiguousarray(np.asarray(inputs["x"]).astype(np_edt))
    maps = []
    for c in range(p.n_cores):
        maps.append({
            "x": x,
            "midx": p.idx_arr[c],
            "smat": p.smat[c].astype(np_edt, copy=False),
            "sdiag": p.sdiag[c].astype(np_edt, copy=False),
            "xown": x[c * p.npc:(c + 1) * p.npc],
            "w1": np.asarray(inputs["W1"], dtype=np.float32),
            "b1": np.asarray(inputs["b1"], dtype=np.float32).reshape(-1, 1),
            "w2": np.asarray(inputs["W2"], dtype=np.float32),
            "b2": np.asarray(inputs["b2"], dtype=np.float32).reshape(-1, 1),
            "w3": np.asarray(inputs["W3"], dtype=np.float32),
            "b3": np.asarray(inputs["b3"], dtype=np.float32).reshape(-1, 1),
            "w4": np.asarray(inputs["W4"], dtype=np.float32),
            "b4": np.asarray(inputs["b4"], dtype=np.float32).reshape(-1, 1),
        })
    return maps


_CACHE = {}


def kernel(_trace=False, **inputs):
    from concourse.bass_utils import run_bass_kernel_spmd

    edge_index = np.asarray(inputs["edge_index"])
    p = make_plan(edge_index)
    key = (p.S, tuple(int(c[3]) for c in p.calls))
    if key not in _CACHE:
        _CACHE[key] = build_program(p)
    nc = _CACHE[key]
    res = run_bass_kernel_spmd(nc, make_in_maps(p, inputs),
                               core_ids=list(range(p.n_cores)),
                               trace=_trace)
    out = np.concatenate([res.results[c]["out"] for c in range(p.n_cores)],
                         axis=0)
    if _trace:
        return out, res
    return out


# revision 20
# speedup vs baseline: 3.1008x; 1.1427x over previous
"""BrainGCN Trainium2 kernel (8 NeuronCores, Bass/Tile).

Model (PyG-style GCNConv x2 + 2 FC layers):
    h = tanh(gcn(x,  W1, b1)); h = tanh(gcn(h, W2, b2))
    h = tanh(h @ W3 + b3);      out = h @ W4 + b4

gcn(x, W, b) = (agg + x * dinv^2) @ W + b  with
    agg[d] = sum_{e:(s,d)} dinv[s]*dinv[d] * x[s]        (by linearity we
aggregate raw feature rows first, then apply W once per node).

Distribution: dst-nodes are split into 8 contiguous blocks (one per core).
Each core aggregates its own dst block; self-loops are folded in as ordinary
edges with coef = dinv^2.  The only cross-core exchange is an AllGather of
the h1 shards between the two GCN layers.

Device-side scatter-add: edges are sorted by dst into 128-wide dst windows.
For every 128-edge tile the DVE builds S[e, d_local] = coef[e] *
onehot(d_local[e]) with a single tensor_scalar (iota == dloc) * coef, and the
PE accumulates aggT[feat, dst] += E_tile^T @ S into PSUM.  Gathers of the
512-byte feature rows run on dma_gather (int16 indices -> the table is
addressed in halves of 25000 rows).
"""

import math

import numpy as np

# ---------------------------------------------------------------- constants
N_NODES = 50000
N_CORES = 8
F_IN, H1D, H2D, H3D, OUTD = 128, 128, 64, 64, 1
WIN = 128          # dst window width (psum free dim of the scatter matmul)
HALF = 25000       # gather-table half size (int16 index range)
G_WINDOWS = 3      # dst windows per gather chunk (per half)
NCHUNK = 512       # fc-layer column chunk


def _cdiv(a, b):
    return -(-a // b)


def _rup(a, b):
    return _cdiv(a, b) * b


# ------------------------------------------------------------------ planning
class Plan:
    pass


def make_plan(edge_index, n_nodes=N_NODES, n_cores=N_CORES, half=HALF,
              g_windows=G_WINDOWS, win=WIN):
    """Host-side graph preprocessing -> static schedule + per-core arrays."""
    src = np.asarray(edge_index[0]).astype(np.int64)
    dst = np.asarray(edge_index[1]).astype(np.int64)

    npc = n_nodes // n_cores
    assert npc * n_cores == n_nodes
    n_win = _cdiv(npc, win)
    n_half = _cdiv(n_nodes, half)
    assert half <= 32767

    deg = np.bincount(dst, minlength=n_nodes).astype(np.float64) + 1.0
    dinv = 1.0 / np.sqrt(deg)

    # self-loops are handled separately (diag matmul); edges only here
    s_all, d_all = src, dst
    coef = (dinv[s_all] * dinv[d_all]).astype(np.float32)

    core = d_all // npc
    w = (d_all % npc) // win
    h = s_all // half

    # per (core, w, h) counts -> static caps shared by all cores
    gid = (core * n_win + w) * n_half + h
    counts = np.bincount(gid, minlength=n_cores * n_win * n_half)
    counts = counts.reshape(n_cores, n_win, n_half)
    caps = counts.max(axis=0)                     # [n_win, n_half] slots

    # window groups (gather chunks)
    wgroups = [list(range(i, min(i + g_windows, n_win)))
               for i in range(0, n_win, g_windows)]

    # static slot offsets, in [wg][h][w] order
    off = {}
    calls = []        # (wg_index, h, slot_off, n_slots)
    pos = 0
    for gi, wg in enumerate(wgroups):
        for hh in range(n_half):
            call_off = pos
            for ww in wg:
                off[(ww, hh)] = pos
                pos += int(caps[ww, hh])
            pos = _rup(pos, 128)                   # trailing pad per call
            calls.append((gi, hh, call_off, pos - call_off))
    S = pos                                        # total slots (mult of 128)
    assert S % 128 == 0
    T = S // 128                                   # total tiles

    # per-window tile lists: [(h, tile_local_in_call, tile_global)]
    win_tiles = []
    for ww in range(n_win):
        tiles = []
        for hh in range(n_half):
            gi = ww // g_windows
            call_off = next(c[2] for c in calls if c[0] == gi and c[1] == hh)
            o, cp = off[(ww, hh)], int(caps[ww, hh])
            if cp == 0:
                continue
            t_first = (o - call_off) // 128
            t_last = (o + cp - 1 - call_off) // 128
            for t in range(t_first, t_last + 1):
                tiles.append((hh, t, call_off // 128 + t))
        win_tiles.append(tiles)

    # ------- per-core arrays
    # order edges: core, then (wg, h, w) -- same as the slot layout
    wg_of_w = np.array([ww // g_windows for ww in range(n_win)])
    order_key = (((core * len(wgroups) + wg_of_w[w]) * n_half + h) * n_win + w)
    order = np.argsort(order_key, kind="stable")
    s_o, d_o, c_o = s_all[order], d_all[order], coef[order]
    core_o, w_o, h_o = core[order], w[order], h[order]

    # destination slot for each edge: static group offset + rank within group.
    # each (core, w, h) group is contiguous in the sorted order; rank = index
    # since the group's first element.
    comb = (core_o * n_win + w_o) * n_half + h_o
    pos = np.arange(comb.size, dtype=np.int64)
    is_start = np.ones(comb.size, dtype=bool)
    if comb.size > 1:
        is_start[1:] = comb[1:] != comb[:-1]
    rank = pos - np.maximum.accumulate(np.where(is_start, pos, 0))

    static_off = np.zeros((n_win, n_half), dtype=np.int64)
    for ww in range(n_win):
        for hh in range(n_half):
            static_off[ww, hh] = off[(ww, hh)]
    slot = static_off[w_o, h_o] + rank

    idx16 = np.zeros((n_cores, S), dtype=np.int16)
    dloc = np.zeros((n_cores, S), dtype=np.float32)
    cof = np.zeros((n_cores, S), dtype=np.float32)
    ci = core_o.astype(np.int64)
    idx16[ci, slot] = (s_o - h_o * half).astype(np.int16)
    dloc[ci, slot] = (d_o - ci * npc - w_o * win).astype(np.float32)
    cof[ci, slot] = c_o

    p = Plan()
    p.n_nodes, p.n_cores, p.npc = n_nodes, n_cores, npc
    p.win, p.n_win, p.half, p.n_half = win, n_win, half, n_half
    p.win_sizes = [min(win, npc - ww * win) for ww in range(n_win)]
    p.wgroups, p.calls, p.win_tiles = wgroups, calls, win_tiles
    p.S, p.T = S, T
    # layouts for the device: idx [128, S/16] (16-row wrap, replicated x8),
    # dloc/coef [128, T] (slot i -> [i%128, i//128])
    p.idx_arr = np.ascontiguousarray(
        np.tile(idx16.reshape(n_cores, S // 16, 16).transpose(0, 2, 1),
                (1, 8, 1)))

    # window owner of each static slot (pad slots: -1)
    w_of_slot = np.full(S, -1, dtype=np.int64)
    for ww in range(n_win):
        for hh in range(n_half):
            o = off[(ww, hh)]
            w_of_slot[o:o + int(caps[ww, hh])] = ww

    # processing-sequence entries: for each wgroup, for each window in it,
    # its win_tiles entries in order.  smat column block q belongs to entry q.
    ents = []                     # (ww, hh, lt, gt)
    wg_ents = []                  # (ent_off, n_ents) per wgroup
    for gi, wg in enumerate(wgroups):
        e0 = len(ents)
        for ww in wg:
            for (hh, lt, gt) in win_tiles[ww]:
                ents.append((ww, hh, lt, gt))
        wg_ents.append((e0, len(ents) - e0))
    p.ents, p.wg_ents = ents, wg_ents
    NE = len(ents)

    # host-built S tiles in sequence order:
    # smat[c, p, q*win + j] = coef if slot gt*128+p belongs to window ww
    smat = np.zeros((n_cores, 128, NE * win), dtype=np.float16)
    dloc_i = dloc.astype(np.int64)
    cidx = np.arange(n_cores)[:, None]
    for q, (ww, hh, lt, gt) in enumerate(ents):
        sl = np.arange(gt * 128, gt * 128 + 128)
        m = w_of_slot[sl] == ww
        rows = np.where(m)[0]
        if rows.size == 0:
            continue
        smat[cidx, rows[None, :],
             q * win + dloc_i[:, sl[m]]] = cof[:, sl[m]]
    p.smat = np.ascontiguousarray(smat)
    p.NE = NE

    # self-loop diagonal: sdiag[c, p, w*win + j] = (p==j)*dinv^2[global node]
    d2 = (dinv * dinv).astype(np.float32)
    sdiag = np.zeros((n_cores, 128, n_win * win), dtype=np.float16)
    for c in range(n_cores):
        for ww in range(n_win):
            wsz = min(win, npc - ww * win)
            g0 = c * npc + ww * win
            sdiag[c, np.arange(wsz), ww * win + np.arange(wsz)] = d2[g0:g0 + wsz]
    p.sdiag = sdiag

    return p


# ------------------------------------------------------------------- program
def build_program(p, debug=False, n_queues=4, scratch=32768, f32r=False,
                  edge_dt="float16"):
    import concourse.bacc as bacc
    import concourse.bass as bass
    import concourse.mybir as mybir
    import concourse.tile as tile
    from concourse.masks import make_identity

    f32 = mybir.dt.float32
    f32r = mybir.dt.float32r if f32r else f32
    edt = getattr(mybir.dt, edge_dt)
    i16 = mybir.dt.int16
    AF = mybir.ActivationFunctionType
    OP = mybir.AluOpType

    nc = bacc.Bacc("TRN2", target_bir_lowering=False, debug=debug,
                   num_devices=p.n_cores, num_swdge_queues=n_queues,
                   dynamic_dma_scratch_size=scratch)

    x_d = nc.dram_tensor("x", [p.n_nodes, F_IN], edt, kind="ExternalInput")
    idx_d = nc.dram_tensor("midx", [128, p.S // 16], i16, kind="ExternalInput")
    smat_d = nc.dram_tensor("smat", [128, p.NE * p.win], edt,
                            kind="ExternalInput")
    sdiag_d = nc.dram_tensor("sdiag", [128, p.n_win * p.win], edt,
                             kind="ExternalInput")
    xown_d = nc.dram_tensor("xown", [p.npc, F_IN], edt, kind="ExternalInput")
    w1_d = nc.dram_tensor("w1", [F_IN, H1D], f32, kind="ExternalInput")
    b1_d = nc.dram_tensor("b1", [H1D, 1], f32, kind="ExternalInput")
    w2_d = nc.dram_tensor("w2", [H1D, H2D], f32, kind="ExternalInput")
    b2_d = nc.dram_tensor("b2", [H2D, 1], f32, kind="ExternalInput")
    w3_d = nc.dram_tensor("w3", [H2D, H3D], f32, kind="ExternalInput")
    b3_d = nc.dram_tensor("b3", [H3D, 1], f32, kind="ExternalInput")
    w4_d = nc.dram_tensor("w4", [H3D, OUTD], f32, kind="ExternalInput")
    b4_d = nc.dram_tensor("b4", [OUTD, 1], f32, kind="ExternalInput")
    out_d = nc.dram_tensor("out", [p.npc, OUTD], f32, kind="ExternalOutput")

    h1_shard = nc.dram_tensor("h1_shard", [p.npc, H1D], edt)
    h1_full = nc.dram_tensor("h1_full", [p.n_nodes, H1D], edt,
                             addr_space="Shared")

    with tile.TileContext(nc) as tc:
        with (
            tc.tile_pool(name="const", bufs=1) as cpool,
            tc.tile_pool(name="gather", bufs=3) as gpool,
            tc.tile_pool(name="sel", bufs=3) as spool,
            tc.tile_pool(name="work", bufs=3) as wpool,
            tc.tile_pool(name="persist", bufs=1) as ppool,
            tc.tile_pool(name="psum", bufs=2, space="PSUM") as pspool,
            tc.tile_pool(name="psumfc", bufs=1, space="PSUM") as pfpool,
        ):
            # ---- constants / metadata to SBUF
            idx_s = cpool.tile([128, p.S // 16], i16)
            nc.sync.dma_start(idx_s[:], idx_d[:, :])
            sdiag_s = cpool.tile([128, p.n_win * p.win], edt)
            nc.sync.dma_start(sdiag_s[:], sdiag_d[:, :])
            w1_s = cpool.tile([F_IN, H1D], f32)
            nc.sync.dma_start(w1_s[:], w1_d[:, :])
            b1_s = cpool.tile([H1D, 1], f32)
            nc.sync.dma_start(b1_s[:], b1_d[:, :])
            w2_s = cpool.tile([H1D, H2D], f32)
            nc.sync.dma_start(w2_s[:], w2_d[:, :])
            b2_s = cpool.tile([H2D, 1], f32)
            nc.sync.dma_start(b2_s[:], b2_d[:, :])
            w3_s = cpool.tile([H2D, H3D], f32)
            nc.sync.dma_start(w3_s[:], w3_d[:, :])
            b3_s = cpool.tile([H3D, 1], f32)
            nc.sync.dma_start(b3_s[:], b3_d[:, :])
            w4_s = cpool.tile([H3D, OUTD], f32)
            nc.sync.dma_start(w4_s[:], w4_d[:, :])
            b4_s = cpool.tile([OUTD, 1], f32)
            nc.sync.dma_start(b4_s[:], b4_d[:, :])
            ident = cpool.tile([128, 128], f32)
            make_identity(nc, ident[:])

            h2T = ppool.tile([H2D, p.npc], f32)

            # ---------------- one GCN layer ----------------
            call_seq = [0]

            def gcn_layer(table_ap_fn, own_ap, w_s, b_s, out_feat, sink):
                for gi, wg in enumerate(p.wgroups):
                    bufs = {}
                    for (cgi, hh, call_off, n_call) in p.calls:
                        if cgi != gi or n_call == 0:
                            continue
                        gb = gpool.tile([128, (n_call // 128) * F_IN], edt,
                                        tag=f"gb{hh}")
                        out3d = gb[:].rearrange("q (t e) -> q t e", e=F_IN)
                        nc.gpsimd.dma_gather(
                            out_ap=out3d,
                            in_ap=table_ap_fn(hh),
                            idxs_ap=idx_s[:, call_off // 16:
                                          (call_off + n_call) // 16],
                            num_idxs=n_call,
                            num_idxs_reg=n_call,
                            elem_size=F_IN,
                            single_packet=False,
                            queue_num=call_seq[0] % n_queues,
                        )
                        call_seq[0] += 1
                        bufs[hh] = gb
                    # stream this window group's host-built S tiles
                    e0g, neg = p.wg_ents[gi]
                    sbf = spool.tile([128, neg * p.win], edt, tag="sbf")
                    nc.sync.dma_start(
                        sbf[:], smat_d[:, e0g * p.win:(e0g + neg) * p.win])
                    nw0 = wg[0] * p.win
                    nrows = min(p.npc, (wg[-1] + 1) * p.win) - nw0
                    xo = wpool.tile([128, len(wg) * F_IN], edt, tag="xo")
                    xo3 = xo[:].rearrange("q (t e) -> q t e", e=F_IN)
                    for j, ww in enumerate(wg):
                        wsz = p.win_sizes[ww]
                        nc.sync.dma_start(
                            xo3[:wsz, j, :],
                            own_ap[ww * p.win: ww * p.win + wsz, :])
                    q = p.wg_ents[gi][0]
                    for ww in wg:
                        tiles = p.win_tiles[ww]
                        wsz = p.win_sizes[ww]
                        j = ww - wg[0]
                        pag = pspool.tile([128, p.win], f32, tag="pag")
                        # self-loop term: x_own[window]^T @ diag(dinv^2)
                        nc.tensor.matmul(
                            pag[:], lhsT=xo[:wsz, j * F_IN:(j + 1) * F_IN],
                            rhs=sdiag_s[:wsz,
                                        ww * p.win: (ww + 1) * p.win],
                            start=True, stop=(not tiles))
                        for k, (hh, lt, gt) in enumerate(tiles):
                            nc.tensor.matmul(
                                pag[:],
                                lhsT=bufs[hh][:, lt * F_IN:(lt + 1) * F_IN],
                                rhs=sbf[:, (q - e0g) * p.win:
                                        (q - e0g + 1) * p.win],
                                start=False, stop=(k == len(tiles) - 1))
                            q += 1
                        aggT = wpool.tile([128, p.win], f32, tag="aggT")
                        nc.vector.tensor_copy(aggT[:], pag[:])
                        ph = pspool.tile([out_feat, p.win], f32, tag="ph")
                        nc.tensor.matmul(ph[:], lhsT=w_s[:], rhs=aggT[:],
                                         start=True, stop=True)
                        sink(ww, wsz, ph, b_s)

            # ---- layer 1: x -> h1_shard (node-major, via PE transpose)
            def sink1(ww, wsz, ph, b_s):
                hT = wpool.tile([128, p.win], f32, tag="hT")
                nc.scalar.activation(hT[:], ph[:], AF.Tanh,
                                     bias=b_s[:, 0:1])
                pt = pspool.tile([128, 128], f32, tag="pt")
                nc.tensor.transpose(pt[:], hT[:], ident[:])
                hw_ = wpool.tile([128, 128], edt, tag="hw")
                nc.vector.tensor_copy(hw_[:], pt[:])
                nc.sync.dma_start(
                    h1_shard[ww * p.win: ww * p.win + wsz, :],
                    hw_[:wsz, :])

            gcn_layer(lambda hh: x_d[hh * p.half:
                                     min((hh + 1) * p.half, p.n_nodes), :],
                      xown_d, w1_s, b1_s, H1D, sink1)

            # ---- exchange h1 shards
            nc.gpsimd.collective_compute(
                "AllGather", mybir.AluOpType.bypass,
                replica_groups=[list(range(p.n_cores))],
                ins=[h1_shard[:, :]], outs=[h1_full[:, :]])

            # ---- layer 2: h1_full -> h2T (kept on-chip, feat-major)
            def sink2(ww, wsz, ph, b_s):
                nc.scalar.activation(
                    h2T[:, ww * p.win: ww * p.win + wsz],
                    ph[:, :wsz], AF.Tanh, bias=b_s[:, 0:1])

            gcn_layer(lambda hh: h1_full[hh * p.half:
                                         min((hh + 1) * p.half, p.n_nodes), :],
                      h1_shard, w2_s, b2_s, H2D, sink2)

            # ---- fc layers on the dst shard
            for c0 in range(0, p.npc, NCHUNK):
                cs = min(NCHUNK, p.npc - c0)
                p3 = pfpool.tile([H3D, NCHUNK], f32, tag="p3")
                nc.tensor.matmul(p3[:, :cs], lhsT=w3_s[:],
                                 rhs=h2T[:, c0:c0 + cs],
                                 start=True, stop=True)
                h3 = wpool.tile([H3D, NCHUNK], f32, tag="h3")
                nc.scalar.activation(h3[:, :cs], p3[:, :cs], AF.Tanh,
                                     bias=b3_s[:, 0:1])
                p4 = pfpool.tile([OUTD, NCHUNK], f32, tag="p4")
                nc.tensor.matmul(p4[:, :cs], lhsT=w4_s[:], rhs=h3[:, :cs],
                                 start=True, stop=True)
                ob = wpool.tile([OUTD, NCHUNK], f32, tag="ob")
                nc.vector.tensor_scalar(
                    out=ob[:, :cs], in0=p4[:, :cs],
                    scalar1=b4_s[0:1, 0:1], scalar2=None, op0=OP.add)
                nc.sync.dma_start(out_d[c0:c0 + cs, :], ob[0:1, :cs])

    nc.compile()
    return nc


def make_in_maps(p, inputs, edge_dt="float16"):
    np_edt = dict(float32=np.float32, float16=np.float16)[edge_dt]
    if edge_dt == "bfloat16":
        import ml_dtypes
        np_edt = ml_dtypes.bfloat16
    x = np.ascontiguousarray(np.asarray(inputs["x"]).astype(np_edt))
    maps = []
    for c in range(p.n_cores):
        maps.append({
            "x": x,
            "midx": p.idx_arr[c],
            "smat": p.smat[c].astype(np_edt, copy=False),
            "sdiag": p.sdiag[c].astype(np_edt, copy=False),
            "xown": x[c * p.npc:(c + 1) * p.npc],
            "w1": np.asarray(inputs["W1"], dtype=np.float32),
            "b1": np.asarray(inputs["b1"], dtype=np.float32).reshape(-1, 1),
            "w2": np.asarray(inputs["W2"], dtype=np.float32),
            "b2": np.asarray(inputs["b2"], dtype=np.float32).reshape(-1, 1),
            "w3": np.asarray(inputs["W3"], dtype=np.float32),
            "b3": np.asarray(inputs["b3"], dtype=np.float32).reshape(-1, 1),
            "w4": np.asarray(inputs["W4"], dtype=np.float32),
            "b4": np.asarray(inputs["b4"], dtype=np.float32).reshape(-1, 1),
        })
    return maps


_CACHE = {}


def kernel(_trace=False, **inputs):
    from concourse.bass_utils import run_bass_kernel_spmd

    edge_index = np.asarray(inputs["edge_index"])
    p = make_plan(edge_index)
    key = (p.S, tuple(int(c[3]) for c in p.calls))
    if key not in _CACHE:
        _CACHE[key] = build_program(p)
    nc = _CACHE[key]
    res = run_bass_kernel_spmd(nc, make_in_maps(p, inputs),
                               core_ids=list(range(p.n_cores)),
                               trace=_trace)
    out = np.concatenate([res.results[c]["out"] for c in range(p.n_cores)],
                         axis=0)
    if _trace:
        return out, res
    return out


# revision 21
# speedup vs baseline: 3.1410x; 1.0130x over previous
"""BrainGCN Trainium2 kernel (8 NeuronCores, Bass/Tile).

Model (PyG-style GCNConv x2 + 2 FC layers):
    h = tanh(gcn(x,  W1, b1)); h = tanh(gcn(h, W2, b2))
    h = tanh(h @ W3 + b3);      out = h @ W4 + b4

gcn(x, W, b) = (agg + x * dinv^2) @ W + b  with
    agg[d] = sum_{e:(s,d)} dinv[s]*dinv[d] * x[s]        (by linearity we
aggregate raw feature rows first, then apply W once per node).

Distribution: dst-nodes are split into 8 contiguous blocks (one per core).
Each core aggregates its own dst block; self-loops are folded in as ordinary
edges with coef = dinv^2.  The only cross-core exchange is an AllGather of
the h1 shards between the two GCN layers.

Device-side scatter-add: edges are sorted by dst into 128-wide dst windows.
For every 128-edge tile the DVE builds S[e, d_local] = coef[e] *
onehot(d_local[e]) with a single tensor_scalar (iota == dloc) * coef, and the
PE accumulates aggT[feat, dst] += E_tile^T @ S into PSUM.  Gathers of the
512-byte feature rows run on dma_gather (int16 indices -> the table is
addressed in halves of 25000 rows).
"""

import math

import numpy as np

# ---------------------------------------------------------------- constants
N_NODES = 50000
N_CORES = 8
F_IN, H1D, H2D, H3D, OUTD = 128, 128, 64, 64, 1
WIN = 128          # dst window width (psum free dim of the scatter matmul)
HALF = 25000       # gather-table half size (int16 index range)
G_WINDOWS = 4      # dst windows per gather chunk (per half)
NCHUNK = 512       # fc-layer column chunk


def _cdiv(a, b):
    return -(-a // b)


def _rup(a, b):
    return _cdiv(a, b) * b


# ------------------------------------------------------------------ planning
class Plan:
    pass


def make_plan(edge_index, n_nodes=N_NODES, n_cores=N_CORES, half=HALF,
              g_windows=G_WINDOWS, win=WIN):
    """Host-side graph preprocessing -> static schedule + per-core arrays."""
    src = np.asarray(edge_index[0]).astype(np.int64)
    dst = np.asarray(edge_index[1]).astype(np.int64)

    npc = n_nodes // n_cores
    assert npc * n_cores == n_nodes
    n_win = _cdiv(npc, win)
    n_half = _cdiv(n_nodes, half)
    assert half <= 32767

    deg = np.bincount(dst, minlength=n_nodes).astype(np.float64) + 1.0
    dinv = 1.0 / np.sqrt(deg)

    # self-loops are handled separately (diag matmul); edges only here
    s_all, d_all = src, dst
    coef = (dinv[s_all] * dinv[d_all]).astype(np.float32)

    core = d_all // npc
    w = (d_all % npc) // win
    h = s_all // half

    # per (core, w, h) counts -> static caps shared by all cores
    gid = (core * n_win + w) * n_half + h
    counts = np.bincount(gid, minlength=n_cores * n_win * n_half)
    counts = counts.reshape(n_cores, n_win, n_half)
    caps = counts.max(axis=0)                     # [n_win, n_half] slots

    # window groups (gather chunks)
    wgroups = [list(range(i, min(i + g_windows, n_win)))
               for i in range(0, n_win, g_windows)]

    # static slot offsets, in [wg][h][w] order
    off = {}
    calls = []        # (wg_index, h, slot_off, n_slots)
    pos = 0
    for gi, wg in enumerate(wgroups):
        for hh in range(n_half):
            call_off = pos
            for ww in wg:
                off[(ww, hh)] = pos
                pos += int(caps[ww, hh])
            pos = _rup(pos, 128)                   # trailing pad per call
            calls.append((gi, hh, call_off, pos - call_off))
    S = pos                                        # total slots (mult of 128)
    assert S % 128 == 0
    T = S // 128                                   # total tiles

    # per-window tile lists: [(h, tile_local_in_call, tile_global)]
    win_tiles = []
    for ww in range(n_win):
        tiles = []
        for hh in range(n_half):
            gi = ww // g_windows
            call_off = next(c[2] for c in calls if c[0] == gi and c[1] == hh)
            o, cp = off[(ww, hh)], int(caps[ww, hh])
            if cp == 0:
                continue
            t_first = (o - call_off) // 128
            t_last = (o + cp - 1 - call_off) // 128
            for t in range(t_first, t_last + 1):
                tiles.append((hh, t, call_off // 128 + t))
        win_tiles.append(tiles)

    # ------- per-core arrays
    # order edges: core, then (wg, h, w) -- same as the slot layout
    wg_of_w = np.array([ww // g_windows for ww in range(n_win)])
    order_key = (((core * len(wgroups) + wg_of_w[w]) * n_half + h) * n_win + w)
    order = np.argsort(order_key, kind="stable")
    s_o, d_o, c_o = s_all[order], d_all[order], coef[order]
    core_o, w_o, h_o = core[order], w[order], h[order]

    # destination slot for each edge: static group offset + rank within group.
    # each (core, w, h) group is contiguous in the sorted order; rank = index
    # since the group's first element.
    comb = (core_o * n_win + w_o) * n_half + h_o
    pos = np.arange(comb.size, dtype=np.int64)
    is_start = np.ones(comb.size, dtype=bool)
    if comb.size > 1:
        is_start[1:] = comb[1:] != comb[:-1]
    rank = pos - np.maximum.accumulate(np.where(is_start, pos, 0))

    static_off = np.zeros((n_win, n_half), dtype=np.int64)
    for ww in range(n_win):
        for hh in range(n_half):
            static_off[ww, hh] = off[(ww, hh)]
    slot = static_off[w_o, h_o] + rank

    idx16 = np.zeros((n_cores, S), dtype=np.int16)
    dloc = np.zeros((n_cores, S), dtype=np.float32)
    cof = np.zeros((n_cores, S), dtype=np.float32)
    ci = core_o.astype(np.int64)
    idx16[ci, slot] = (s_o - h_o * half).astype(np.int16)
    dloc[ci, slot] = (d_o - ci * npc - w_o * win).astype(np.float32)
    cof[ci, slot] = c_o

    p = Plan()
    p.n_nodes, p.n_cores, p.npc = n_nodes, n_cores, npc
    p.win, p.n_win, p.half, p.n_half = win, n_win, half, n_half
    p.win_sizes = [min(win, npc - ww * win) for ww in range(n_win)]
    p.wgroups, p.calls, p.win_tiles = wgroups, calls, win_tiles
    p.S, p.T = S, T
    # layouts for the device: idx [128, S/16] (16-row wrap, replicated x8),
    # dloc/coef [128, T] (slot i -> [i%128, i//128])
    p.idx_arr = np.ascontiguousarray(
        np.tile(idx16.reshape(n_cores, S // 16, 16).transpose(0, 2, 1),
                (1, 8, 1)))

    # window owner of each static slot (pad slots: -1)
    w_of_slot = np.full(S, -1, dtype=np.int64)
    for ww in range(n_win):
        for hh in range(n_half):
            o = off[(ww, hh)]
            w_of_slot[o:o + int(caps[ww, hh])] = ww

    # processing-sequence entries: for each wgroup, for each window in it,
    # its win_tiles entries in order.  smat column block q belongs to entry q.
    ents = []                     # (ww, hh, lt, gt)
    wg_ents = []                  # (ent_off, n_ents) per wgroup
    for gi, wg in enumerate(wgroups):
        e0 = len(ents)
        for ww in wg:
            for (hh, lt, gt) in win_tiles[ww]:
                ents.append((ww, hh, lt, gt))
        wg_ents.append((e0, len(ents) - e0))
    p.ents, p.wg_ents = ents, wg_ents
    NE = len(ents)

    # host-built S tiles in sequence order:
    # smat[c, p, q*win + j] = coef if slot gt*128+p belongs to window ww
    smat = np.zeros((n_cores, 128, NE * win), dtype=np.float16)
    dloc_i = dloc.astype(np.int64)
    cidx = np.arange(n_cores)[:, None]
    for q, (ww, hh, lt, gt) in enumerate(ents):
        sl = np.arange(gt * 128, gt * 128 + 128)
        m = w_of_slot[sl] == ww
        rows = np.where(m)[0]
        if rows.size == 0:
            continue
        smat[cidx, rows[None, :],
             q * win + dloc_i[:, sl[m]]] = cof[:, sl[m]]
    p.smat = np.ascontiguousarray(smat)
    p.NE = NE

    # self-loop diagonal: sdiag[c, p, w*win + j] = (p==j)*dinv^2[global node]
    d2 = (dinv * dinv).astype(np.float32)
    sdiag = np.zeros((n_cores, 128, n_win * win), dtype=np.float16)
    for c in range(n_cores):
        for ww in range(n_win):
            wsz = min(win, npc - ww * win)
            g0 = c * npc + ww * win
            sdiag[c, np.arange(wsz), ww * win + np.arange(wsz)] = d2[g0:g0 + wsz]
    p.sdiag = sdiag

    return p


# ------------------------------------------------------------------- program
def build_program(p, debug=False, n_queues=4, scratch=32768, f32r=False,
                  edge_dt="float16"):
    import concourse.bacc as bacc
    import concourse.bass as bass
    import concourse.mybir as mybir
    import concourse.tile as tile
    from concourse.masks import make_identity

    f32 = mybir.dt.float32
    f32r = mybir.dt.float32r if f32r else f32
    edt = getattr(mybir.dt, edge_dt)
    i16 = mybir.dt.int16
    AF = mybir.ActivationFunctionType
    OP = mybir.AluOpType

    nc = bacc.Bacc("TRN2", target_bir_lowering=False, debug=debug,
                   num_devices=p.n_cores, num_swdge_queues=n_queues,
                   dynamic_dma_scratch_size=scratch)

    x_d = nc.dram_tensor("x", [p.n_nodes, F_IN], edt, kind="ExternalInput")
    idx_d = nc.dram_tensor("midx", [128, p.S // 16], i16, kind="ExternalInput")
    smat_d = nc.dram_tensor("smat", [128, p.NE * p.win], edt,
                            kind="ExternalInput")
    sdiag_d = nc.dram_tensor("sdiag", [128, p.n_win * p.win], edt,
                             kind="ExternalInput")
    xown_d = nc.dram_tensor("xown", [p.npc, F_IN], edt, kind="ExternalInput")
    w1_d = nc.dram_tensor("w1", [F_IN, H1D], f32, kind="ExternalInput")
    b1_d = nc.dram_tensor("b1", [H1D, 1], f32, kind="ExternalInput")
    w2_d = nc.dram_tensor("w2", [H1D, H2D], f32, kind="ExternalInput")
    b2_d = nc.dram_tensor("b2", [H2D, 1], f32, kind="ExternalInput")
    w3_d = nc.dram_tensor("w3", [H2D, H3D], f32, kind="ExternalInput")
    b3_d = nc.dram_tensor("b3", [H3D, 1], f32, kind="ExternalInput")
    w4_d = nc.dram_tensor("w4", [H3D, OUTD], f32, kind="ExternalInput")
    b4_d = nc.dram_tensor("b4", [OUTD, 1], f32, kind="ExternalInput")
    out_d = nc.dram_tensor("out", [p.npc, OUTD], f32, kind="ExternalOutput")

    h1_shard = nc.dram_tensor("h1_shard", [p.npc, H1D], edt)
    h1_full = nc.dram_tensor("h1_full", [p.n_nodes, H1D], edt,
                             addr_space="Shared")

    with tile.TileContext(nc) as tc:
        with (
            tc.tile_pool(name="const", bufs=1) as cpool,
            tc.tile_pool(name="gather", bufs=3) as gpool,
            tc.tile_pool(name="sel", bufs=3) as spool,
            tc.tile_pool(name="work", bufs=3) as wpool,
            tc.tile_pool(name="persist", bufs=1) as ppool,
            tc.tile_pool(name="psum", bufs=2, space="PSUM") as pspool,
            tc.tile_pool(name="psumfc", bufs=1, space="PSUM") as pfpool,
        ):
            # ---- constants / metadata to SBUF
            idx_s = cpool.tile([128, p.S // 16], i16)
            nc.sync.dma_start(idx_s[:], idx_d[:, :])
            sdiag_s = cpool.tile([128, p.n_win * p.win], edt)
            nc.sync.dma_start(sdiag_s[:], sdiag_d[:, :])
            w1_s = cpool.tile([F_IN, H1D], f32)
            nc.sync.dma_start(w1_s[:], w1_d[:, :])
            b1_s = cpool.tile([H1D, 1], f32)
            nc.sync.dma_start(b1_s[:], b1_d[:, :])
            w2_s = cpool.tile([H1D, H2D], f32)
            nc.sync.dma_start(w2_s[:], w2_d[:, :])
            b2_s = cpool.tile([H2D, 1], f32)
            nc.sync.dma_start(b2_s[:], b2_d[:, :])
            w3_s = cpool.tile([H2D, H3D], f32)
            nc.sync.dma_start(w3_s[:], w3_d[:, :])
            b3_s = cpool.tile([H3D, 1], f32)
            nc.sync.dma_start(b3_s[:], b3_d[:, :])
            w4_s = cpool.tile([H3D, OUTD], f32)
            nc.sync.dma_start(w4_s[:], w4_d[:, :])
            b4_s = cpool.tile([OUTD, 1], f32)
            nc.sync.dma_start(b4_s[:], b4_d[:, :])
            ident = cpool.tile([128, 128], f32)
            make_identity(nc, ident[:])

            h2T = ppool.tile([H2D, p.npc], f32)

            # ---------------- one GCN layer ----------------
            call_seq = [0]

            def gcn_layer(table_ap_fn, own_ap, w_s, b_s, out_feat, sink):
                for gi, wg in enumerate(p.wgroups):
                    bufs = {}
                    for (cgi, hh, call_off, n_call) in p.calls:
                        if cgi != gi or n_call == 0:
                            continue
                        gb = gpool.tile([128, (n_call // 128) * F_IN], edt,
                                        tag=f"gb{hh}")
                        out3d = gb[:].rearrange("q (t e) -> q t e", e=F_IN)
                        nc.gpsimd.dma_gather(
                            out_ap=out3d,
                            in_ap=table_ap_fn(hh),
                            idxs_ap=idx_s[:, call_off // 16:
                                          (call_off + n_call) // 16],
                            num_idxs=n_call,
                            num_idxs_reg=n_call,
                            elem_size=F_IN,
                            single_packet=False,
                            queue_num=call_seq[0] % n_queues,
                        )
                        call_seq[0] += 1
                        bufs[hh] = gb
                    # stream this window group's host-built S tiles
                    e0g, neg = p.wg_ents[gi]
                    sbf = spool.tile([128, neg * p.win], edt, tag="sbf")
                    nc.sync.dma_start(
                        sbf[:], smat_d[:, e0g * p.win:(e0g + neg) * p.win])
                    nw0 = wg[0] * p.win
                    nrows = min(p.npc, (wg[-1] + 1) * p.win) - nw0
                    xo = wpool.tile([128, len(wg) * F_IN], edt, tag="xo")
                    xo3 = xo[:].rearrange("q (t e) -> q t e", e=F_IN)
                    for j, ww in enumerate(wg):
                        wsz = p.win_sizes[ww]
                        nc.sync.dma_start(
                            xo3[:wsz, j, :],
                            own_ap[ww * p.win: ww * p.win + wsz, :])
                    q = p.wg_ents[gi][0]
                    for ww in wg:
                        tiles = p.win_tiles[ww]
                        wsz = p.win_sizes[ww]
                        j = ww - wg[0]
                        pag = pspool.tile([128, p.win], f32, tag="pag")
                        # self-loop term: x_own[window]^T @ diag(dinv^2)
                        nc.tensor.matmul(
                            pag[:], lhsT=xo[:wsz, j * F_IN:(j + 1) * F_IN],
                            rhs=sdiag_s[:wsz,
                                        ww * p.win: (ww + 1) * p.win],
                            start=True, stop=(not tiles))
                        for k, (hh, lt, gt) in enumerate(tiles):
                            nc.tensor.matmul(
                                pag[:],
                                lhsT=bufs[hh][:, lt * F_IN:(lt + 1) * F_IN],
                                rhs=sbf[:, (q - e0g) * p.win:
                                        (q - e0g + 1) * p.win],
                                start=False, stop=(k == len(tiles) - 1))
                            q += 1
                        aggT = wpool.tile([128, p.win], f32, tag="aggT")
                        nc.vector.tensor_copy(aggT[:], pag[:])
                        ph = pspool.tile([out_feat, p.win], f32, tag="ph")
                        nc.tensor.matmul(ph[:], lhsT=w_s[:], rhs=aggT[:],
                                         start=True, stop=True)
                        sink(ww, wsz, ph, b_s)

            # ---- layer 1: x -> h1_shard (node-major, via PE transpose)
            def sink1(ww, wsz, ph, b_s):
                hT = wpool.tile([128, p.win], f32, tag="hT")
                nc.scalar.activation(hT[:], ph[:], AF.Tanh,
                                     bias=b_s[:, 0:1])
                pt = pspool.tile([128, 128], f32, tag="pt")
                nc.tensor.transpose(pt[:], hT[:], ident[:])
                hw_ = wpool.tile([128, 128], edt, tag="hw")
                nc.vector.tensor_copy(hw_[:], pt[:])
                nc.sync.dma_start(
                    h1_shard[ww * p.win: ww * p.win + wsz, :],
                    hw_[:wsz, :])

            gcn_layer(lambda hh: x_d[hh * p.half:
                                     min((hh + 1) * p.half, p.n_nodes), :],
                      xown_d, w1_s, b1_s, H1D, sink1)

            # ---- exchange h1 shards
            nc.gpsimd.collective_compute(
                "AllGather", mybir.AluOpType.bypass,
                replica_groups=[list(range(p.n_cores))],
                ins=[h1_shard[:, :]], outs=[h1_full[:, :]])

            # ---- layer 2: h1_full -> h2T (kept on-chip, feat-major)
            def sink2(ww, wsz, ph, b_s):
                nc.scalar.activation(
                    h2T[:, ww * p.win: ww * p.win + wsz],
                    ph[:, :wsz], AF.Tanh, bias=b_s[:, 0:1])

            gcn_layer(lambda hh: h1_full[hh * p.half:
                                         min((hh + 1) * p.half, p.n_nodes), :],
                      h1_shard, w2_s, b2_s, H2D, sink2)

            # ---- fc layers on the dst shard
            for c0 in range(0, p.npc, NCHUNK):
                cs = min(NCHUNK, p.npc - c0)
                p3 = pfpool.tile([H3D, NCHUNK], f32, tag="p3")
                nc.tensor.matmul(p3[:, :cs], lhsT=w3_s[:],
                                 rhs=h2T[:, c0:c0 + cs],
                                 start=True, stop=True)
                h3 = wpool.tile([H3D, NCHUNK], f32, tag="h3")
                nc.scalar.activation(h3[:, :cs], p3[:, :cs], AF.Tanh,
                                     bias=b3_s[:, 0:1])
                p4 = pfpool.tile([OUTD, NCHUNK], f32, tag="p4")
                nc.tensor.matmul(p4[:, :cs], lhsT=w4_s[:], rhs=h3[:, :cs],
                                 start=True, stop=True)
                ob = wpool.tile([OUTD, NCHUNK], f32, tag="ob")
                nc.vector.tensor_scalar(
                    out=ob[:, :cs], in0=p4[:, :cs],
                    scalar1=b4_s[0:1, 0:1], scalar2=None, op0=OP.add)
                nc.sync.dma_start(out_d[c0:c0 + cs, :], ob[0:1, :cs])

    nc.compile()
    return nc


def make_in_maps(p, inputs, edge_dt="float16"):
    np_edt = dict(float32=np.float32, float16=np.float16)[edge_dt]
    if edge_dt == "bfloat16":
        import ml_dtypes
        np_edt = ml_dtypes.bfloat16
    x = np.ascontiguousarray(np.asarray(inputs["x"]).astype(np_edt))
    maps = []
    for c in range(p.n_cores):
        maps.append({
            "x": x,
            "midx": p.idx_arr[c],
            "smat": p.smat[c].astype(np_edt, copy=False),
            "sdiag": p.sdiag[c].astype(np_edt, copy=False),
            "xown": x[c * p.npc:(c + 1) * p.npc],
            "w1": np.asarray(inputs["W1"], dtype=np.float32),
            "b1": np.asarray(inputs["b1"], dtype=np.float32).reshape(-1, 1),
            "w2": np.asarray(inputs["W2"], dtype=np.float32),
            "b2": np.asarray(inputs["b2"], dtype=np.float32).reshape(-1, 1),
            "w3": np.asarray(inputs["W3"], dtype=np.float32),
            "b3": np.asarray(inputs["b3"], dtype=np.float32).reshape(-1, 1),
            "w4": np.asarray(inputs["W4"], dtype=np.float32),
            "b4": np.asarray(inputs["b4"], dtype=np.float32).reshape(-1, 1),
        })
    return maps


_CACHE = {}


def kernel(_trace=False, **inputs):
    from concourse.bass_utils import run_bass_kernel_spmd

    edge_index = np.asarray(inputs["edge_index"])
    p = make_plan(edge_index)
    key = (p.S, tuple(int(c[3]) for c in p.calls))
    if key not in _CACHE:
        _CACHE[key] = build_program(p)
    nc = _CACHE[key]
    res = run_bass_kernel_spmd(nc, make_in_maps(p, inputs),
                               core_ids=list(range(p.n_cores)),
                               trace=_trace)
    out = np.concatenate([res.results[c]["out"] for c in range(p.n_cores)],
                         axis=0)
    if _trace:
        return out, res
    return out


# revision 23
# speedup vs baseline: 3.1627x; 1.0069x over previous
"""BrainGCN Trainium2 kernel (8 NeuronCores, Bass/Tile).

Model (PyG-style GCNConv x2 + 2 FC layers):
    h = tanh(gcn(x,  W1, b1)); h = tanh(gcn(h, W2, b2))
    h = tanh(h @ W3 + b3);      out = h @ W4 + b4

gcn(x, W, b) = (agg + x * dinv^2) @ W + b  with
    agg[d] = sum_{e:(s,d)} dinv[s]*dinv[d] * x[s]        (by linearity we
aggregate raw feature rows first, then apply W once per node).

Distribution: dst-nodes are split into 8 contiguous blocks (one per core).
Each core aggregates its own dst block; self-loops are folded in as ordinary
edges with coef = dinv^2.  The only cross-core exchange is an AllGather of
the h1 shards between the two GCN layers.

Device-side scatter-add: edges are sorted by dst into 128-wide dst windows.
For every 128-edge tile the DVE builds S[e, d_local] = coef[e] *
onehot(d_local[e]) with a single tensor_scalar (iota == dloc) * coef, and the
PE accumulates aggT[feat, dst] += E_tile^T @ S into PSUM.  Gathers of the
512-byte feature rows run on dma_gather (int16 indices -> the table is
addressed in halves of 25000 rows).
"""

import math

import numpy as np

# ---------------------------------------------------------------- constants
N_NODES = 50000
N_CORES = 8
F_IN, H1D, H2D, H3D, OUTD = 128, 128, 64, 64, 1
WIN = 128          # dst window width (psum free dim of the scatter matmul)
HALF = 25000       # gather-table half size (int16 index range)
G_WINDOWS = 5      # dst windows per gather chunk (per half)
NCHUNK = 512       # fc-layer column chunk


def _cdiv(a, b):
    return -(-a // b)


def _rup(a, b):
    return _cdiv(a, b) * b


# ------------------------------------------------------------------ planning
class Plan:
    pass


def make_plan(edge_index, n_nodes=N_NODES, n_cores=N_CORES, half=HALF,
              g_windows=G_WINDOWS, win=WIN):
    """Host-side graph preprocessing -> static schedule + per-core arrays."""
    src = np.asarray(edge_index[0]).astype(np.int64)
    dst = np.asarray(edge_index[1]).astype(np.int64)

    npc = n_nodes // n_cores
    assert npc * n_cores == n_nodes
    n_win = _cdiv(npc, win)
    n_half = _cdiv(n_nodes, half)
    assert half <= 32767

    deg = np.bincount(dst, minlength=n_nodes).astype(np.float64) + 1.0
    dinv = 1.0 / np.sqrt(deg)

    # self-loops are handled separately (diag matmul); edges only here
    s_all, d_all = src, dst
    coef = (dinv[s_all] * dinv[d_all]).astype(np.float32)

    core = d_all // npc
    w = (d_all % npc) // win
    h = s_all // half

    # per (core, w, h) counts -> static caps shared by all cores
    gid = (core * n_win + w) * n_half + h
    counts = np.bincount(gid, minlength=n_cores * n_win * n_half)
    counts = counts.reshape(n_cores, n_win, n_half)
    caps = counts.max(axis=0)                     # [n_win, n_half] slots

    # window groups (gather chunks)
    wgroups = [list(range(i, min(i + g_windows, n_win)))
               for i in range(0, n_win, g_windows)]

    # static slot offsets, in [wg][h][w] order
    off = {}
    calls = []        # (wg_index, h, slot_off, n_slots)
    pos = 0
    for gi, wg in enumerate(wgroups):
        for hh in range(n_half):
            call_off = pos
            for ww in wg:
                off[(ww, hh)] = pos
                pos += int(caps[ww, hh])
            pos = _rup(pos, 128)                   # trailing pad per call
            calls.append((gi, hh, call_off, pos - call_off))
    S = pos                                        # total slots (mult of 128)
    assert S % 128 == 0
    T = S // 128                                   # total tiles

    # per-window tile lists: [(h, tile_local_in_call, tile_global)]
    win_tiles = []
    for ww in range(n_win):
        tiles = []
        for hh in range(n_half):
            gi = ww // g_windows
            call_off = next(c[2] for c in calls if c[0] == gi and c[1] == hh)
            o, cp = off[(ww, hh)], int(caps[ww, hh])
            if cp == 0:
                continue
            t_first = (o - call_off) // 128
            t_last = (o + cp - 1 - call_off) // 128
            for t in range(t_first, t_last + 1):
                tiles.append((hh, t, call_off // 128 + t))
        win_tiles.append(tiles)

    # ------- per-core arrays
    # order edges: core, then (wg, h, w) -- same as the slot layout
    wg_of_w = np.array([ww // g_windows for ww in range(n_win)])
    order_key = (((core * len(wgroups) + wg_of_w[w]) * n_half + h) * n_win + w)
    order = np.argsort(order_key, kind="stable")
    s_o, d_o, c_o = s_all[order], d_all[order], coef[order]
    core_o, w_o, h_o = core[order], w[order], h[order]

    # destination slot for each edge: static group offset + rank within group.
    # each (core, w, h) group is contiguous in the sorted order; rank = index
    # since the group's first element.
    comb = (core_o * n_win + w_o) * n_half + h_o
    pos = np.arange(comb.size, dtype=np.int64)
    is_start = np.ones(comb.size, dtype=bool)
    if comb.size > 1:
        is_start[1:] = comb[1:] != comb[:-1]
    rank = pos - np.maximum.accumulate(np.where(is_start, pos, 0))

    static_off = np.zeros((n_win, n_half), dtype=np.int64)
    for ww in range(n_win):
        for hh in range(n_half):
            static_off[ww, hh] = off[(ww, hh)]
    slot = static_off[w_o, h_o] + rank

    idx16 = np.zeros((n_cores, S), dtype=np.int16)
    dloc = np.zeros((n_cores, S), dtype=np.float32)
    cof = np.zeros((n_cores, S), dtype=np.float32)
    ci = core_o.astype(np.int64)
    idx16[ci, slot] = (s_o - h_o * half).astype(np.int16)
    dloc[ci, slot] = (d_o - ci * npc - w_o * win).astype(np.float32)
    cof[ci, slot] = c_o

    p = Plan()
    p.n_nodes, p.n_cores, p.npc = n_nodes, n_cores, npc
    p.win, p.n_win, p.half, p.n_half = win, n_win, half, n_half
    p.win_sizes = [min(win, npc - ww * win) for ww in range(n_win)]
    p.wgroups, p.calls, p.win_tiles = wgroups, calls, win_tiles
    p.S, p.T = S, T
    # layouts for the device: idx [128, S/16] (16-row wrap, replicated x8),
    # dloc/coef [128, T] (slot i -> [i%128, i//128])
    p.idx_arr = np.ascontiguousarray(
        np.tile(idx16.reshape(n_cores, S // 16, 16).transpose(0, 2, 1),
                (1, 8, 1)))

    # window owner of each static slot (pad slots: -1)
    w_of_slot = np.full(S, -1, dtype=np.int64)
    for ww in range(n_win):
        for hh in range(n_half):
            o = off[(ww, hh)]
            w_of_slot[o:o + int(caps[ww, hh])] = ww

    # processing-sequence entries: for each wgroup, for each window in it,
    # its win_tiles entries in order.  smat column block q belongs to entry q.
    ents = []                     # (ww, hh, lt, gt)
    wg_ents = []                  # (ent_off, n_ents) per wgroup
    for gi, wg in enumerate(wgroups):
        e0 = len(ents)
        for ww in wg:
            for (hh, lt, gt) in win_tiles[ww]:
                ents.append((ww, hh, lt, gt))
        wg_ents.append((e0, len(ents) - e0))
    p.ents, p.wg_ents = ents, wg_ents
    NE = len(ents)

    # host-built S tiles in sequence order:
    # smat[c, p, q*win + j] = coef if slot gt*128+p belongs to window ww
    smat = np.zeros((n_cores, 128, NE * win), dtype=np.float16)
    dloc_i = dloc.astype(np.int64)
    cidx = np.arange(n_cores)[:, None]
    for q, (ww, hh, lt, gt) in enumerate(ents):
        sl = np.arange(gt * 128, gt * 128 + 128)
        m = w_of_slot[sl] == ww
        rows = np.where(m)[0]
        if rows.size == 0:
            continue
        smat[cidx, rows[None, :],
             q * win + dloc_i[:, sl[m]]] = cof[:, sl[m]]
    p.smat = np.ascontiguousarray(smat)
    p.NE = NE

    # self-loop diagonal: sdiag[c, p, w*win + j] = (p==j)*dinv^2[global node]
    d2 = (dinv * dinv).astype(np.float32)
    sdiag = np.zeros((n_cores, 128, n_win * win), dtype=np.float16)
    for c in range(n_cores):
        for ww in range(n_win):
            wsz = min(win, npc - ww * win)
            g0 = c * npc + ww * win
            sdiag[c, np.arange(wsz), ww * win + np.arange(wsz)] = d2[g0:g0 + wsz]
    p.sdiag = sdiag

    return p


# ------------------------------------------------------------------- program
def build_program(p, debug=False, n_queues=4, scratch=32768, f32r=False,
                  edge_dt="float16"):
    import concourse.bacc as bacc
    import concourse.bass as bass
    import concourse.mybir as mybir
    import concourse.tile as tile
    from concourse.masks import make_identity

    f32 = mybir.dt.float32
    f32r = mybir.dt.float32r if f32r else f32
    edt = getattr(mybir.dt, edge_dt)
    i16 = mybir.dt.int16
    AF = mybir.ActivationFunctionType
    OP = mybir.AluOpType

    nc = bacc.Bacc("TRN2", target_bir_lowering=False, debug=debug,
                   num_devices=p.n_cores, num_swdge_queues=n_queues,
                   dynamic_dma_scratch_size=scratch)

    x_d = nc.dram_tensor("x", [p.n_nodes, F_IN], edt, kind="ExternalInput")
    idx_d = nc.dram_tensor("midx", [128, p.S // 16], i16, kind="ExternalInput")
    smat_d = nc.dram_tensor("smat", [128, p.NE * p.win], edt,
                            kind="ExternalInput")
    sdiag_d = nc.dram_tensor("sdiag", [128, p.n_win * p.win], edt,
                             kind="ExternalInput")
    xown_d = nc.dram_tensor("xown", [p.npc, F_IN], edt, kind="ExternalInput")
    w1_d = nc.dram_tensor("w1", [F_IN, H1D], f32, kind="ExternalInput")
    b1_d = nc.dram_tensor("b1", [H1D, 1], f32, kind="ExternalInput")
    w2_d = nc.dram_tensor("w2", [H1D, H2D], f32, kind="ExternalInput")
    b2_d = nc.dram_tensor("b2", [H2D, 1], f32, kind="ExternalInput")
    w3_d = nc.dram_tensor("w3", [H2D, H3D], f32, kind="ExternalInput")
    b3_d = nc.dram_tensor("b3", [H3D, 1], f32, kind="ExternalInput")
    w4_d = nc.dram_tensor("w4", [H3D, OUTD], f32, kind="ExternalInput")
    b4_d = nc.dram_tensor("b4", [OUTD, 1], f32, kind="ExternalInput")
    out_d = nc.dram_tensor("out", [p.npc, OUTD], f32, kind="ExternalOutput")

    h1_shard = nc.dram_tensor("h1_shard", [p.npc, H1D], edt)
    h1_full = nc.dram_tensor("h1_full", [p.n_nodes, H1D], edt,
                             addr_space="Shared")

    with tile.TileContext(nc) as tc:
        with (
            tc.tile_pool(name="const", bufs=1) as cpool,
            tc.tile_pool(name="gather", bufs=3) as gpool,
            tc.tile_pool(name="sel", bufs=2) as spool,
            tc.tile_pool(name="work", bufs=3) as wpool,
            tc.tile_pool(name="persist", bufs=1) as ppool,
            tc.tile_pool(name="psum", bufs=2, space="PSUM") as pspool,
            tc.tile_pool(name="psumfc", bufs=1, space="PSUM") as pfpool,
        ):
            # ---- constants / metadata to SBUF
            idx_s = cpool.tile([128, p.S // 16], i16)
            nc.sync.dma_start(idx_s[:], idx_d[:, :])
            sdiag_s = cpool.tile([128, p.n_win * p.win], edt)
            nc.sync.dma_start(sdiag_s[:], sdiag_d[:, :])
            w1_s = cpool.tile([F_IN, H1D], f32)
            nc.sync.dma_start(w1_s[:], w1_d[:, :])
            b1_s = cpool.tile([H1D, 1], f32)
            nc.sync.dma_start(b1_s[:], b1_d[:, :])
            w2_s = cpool.tile([H1D, H2D], f32)
            nc.sync.dma_start(w2_s[:], w2_d[:, :])
            b2_s = cpool.tile([H2D, 1], f32)
            nc.sync.dma_start(b2_s[:], b2_d[:, :])
            w3_s = cpool.tile([H2D, H3D], f32)
            nc.sync.dma_start(w3_s[:], w3_d[:, :])
            b3_s = cpool.tile([H3D, 1], f32)
            nc.sync.dma_start(b3_s[:], b3_d[:, :])
            w4_s = cpool.tile([H3D, OUTD], f32)
            nc.sync.dma_start(w4_s[:], w4_d[:, :])
            b4_s = cpool.tile([OUTD, 1], f32)
            nc.sync.dma_start(b4_s[:], b4_d[:, :])
            ident = cpool.tile([128, 128], f32)
            make_identity(nc, ident[:])

            h2T = ppool.tile([H2D, p.npc], f32)

            # ---------------- one GCN layer ----------------
            call_seq = [0]

            def gcn_layer(table_ap_fn, own_ap, w_s, b_s, out_feat, sink):
                for gi, wg in enumerate(p.wgroups):
                    bufs = {}
                    for (cgi, hh, call_off, n_call) in p.calls:
                        if cgi != gi or n_call == 0:
                            continue
                        gb = gpool.tile([128, (n_call // 128) * F_IN], edt,
                                        tag=f"gb{hh}")
                        out3d = gb[:].rearrange("q (t e) -> q t e", e=F_IN)
                        nc.gpsimd.dma_gather(
                            out_ap=out3d,
                            in_ap=table_ap_fn(hh),
                            idxs_ap=idx_s[:, call_off // 16:
                                          (call_off + n_call) // 16],
                            num_idxs=n_call,
                            num_idxs_reg=n_call,
                            elem_size=F_IN,
                            single_packet=False,
                            queue_num=call_seq[0] % n_queues,
                        )
                        call_seq[0] += 1
                        bufs[hh] = gb
                    # stream this window group's host-built S tiles
                    e0g, neg = p.wg_ents[gi]
                    sbf = spool.tile([128, neg * p.win], edt, tag="sbf")
                    nc.sync.dma_start(
                        sbf[:], smat_d[:, e0g * p.win:(e0g + neg) * p.win])
                    nw0 = wg[0] * p.win
                    nrows = min(p.npc, (wg[-1] + 1) * p.win) - nw0
                    xo = wpool.tile([128, len(wg) * F_IN], edt, tag="xo")
                    xo3 = xo[:].rearrange("q (t e) -> q t e", e=F_IN)
                    for j, ww in enumerate(wg):
                        wsz = p.win_sizes[ww]
                        nc.sync.dma_start(
                            xo3[:wsz, j, :],
                            own_ap[ww * p.win: ww * p.win + wsz, :])
                    q = p.wg_ents[gi][0]
                    for ww in wg:
                        tiles = p.win_tiles[ww]
                        wsz = p.win_sizes[ww]
                        j = ww - wg[0]
                        pag = pspool.tile([128, p.win], f32, tag="pag")
                        # self-loop term: x_own[window]^T @ diag(dinv^2)
                        nc.tensor.matmul(
                            pag[:], lhsT=xo[:wsz, j * F_IN:(j + 1) * F_IN],
                            rhs=sdiag_s[:wsz,
                                        ww * p.win: (ww + 1) * p.win],
                            start=True, stop=(not tiles))
                        for k, (hh, lt, gt) in enumerate(tiles):
                            nc.tensor.matmul(
                                pag[:],
                                lhsT=bufs[hh][:, lt * F_IN:(lt + 1) * F_IN],
                                rhs=sbf[:, (q - e0g) * p.win:
                                        (q - e0g + 1) * p.win],
                                start=False, stop=(k == len(tiles) - 1))
                            q += 1
                        aggT = wpool.tile([128, p.win], f32, tag="aggT")
                        nc.vector.tensor_copy(aggT[:], pag[:])
                        ph = pspool.tile([out_feat, p.win], f32, tag="ph")
                        nc.tensor.matmul(ph[:], lhsT=w_s[:], rhs=aggT[:],
                                         start=True, stop=True)
                        sink(ww, wsz, ph, b_s)

            # ---- layer 1: x -> h1_shard (node-major, via PE transpose)
            def sink1(ww, wsz, ph, b_s):
                hT = wpool.tile([128, p.win], f32, tag="hT")
                nc.scalar.activation(hT[:], ph[:], AF.Tanh,
                                     bias=b_s[:, 0:1])
                pt = pspool.tile([128, 128], f32, tag="pt")
                nc.tensor.transpose(pt[:], hT[:], ident[:])
                hw_ = wpool.tile([128, 128], edt, tag="hw")
                nc.vector.tensor_copy(hw_[:], pt[:])
                nc.sync.dma_start(
                    h1_shard[ww * p.win: ww * p.win + wsz, :],
                    hw_[:wsz, :])

            gcn_layer(lambda hh: x_d[hh * p.half:
                                     min((hh + 1) * p.half, p.n_nodes), :],
                      xown_d, w1_s, b1_s, H1D, sink1)

            # ---- exchange h1 shards
            nc.gpsimd.collective_compute(
                "AllGather", mybir.AluOpType.bypass,
                replica_groups=[list(range(p.n_cores))],
                ins=[h1_shard[:, :]], outs=[h1_full[:, :]])

            # ---- layer 2: h1_full -> h2T (kept on-chip, feat-major)
            def sink2(ww, wsz, ph, b_s):
                nc.scalar.activation(
                    h2T[:, ww * p.win: ww * p.win + wsz],
                    ph[:, :wsz], AF.Tanh, bias=b_s[:, 0:1])

            gcn_layer(lambda hh: h1_full[hh * p.half:
                                         min((hh + 1) * p.half, p.n_nodes), :],
                      h1_shard, w2_s, b2_s, H2D, sink2)

            # ---- fc layers on the dst shard
            for c0 in range(0, p.npc, NCHUNK):
                cs = min(NCHUNK, p.npc - c0)
                p3 = pfpool.tile([H3D, NCHUNK], f32, tag="p3")
                nc.tensor.matmul(p3[:, :cs], lhsT=w3_s[:],
                                 rhs=h2T[:, c0:c0 + cs],
                                 start=True, stop=True)
                h3 = wpool.tile([H3D, NCHUNK], f32, tag="h3")
                nc.scalar.activation(h3[:, :cs], p3[:, :cs], AF.Tanh,
                                     bias=b3_s[:, 0:1])
                p4 = pfpool.tile([OUTD, NCHUNK], f32, tag="p4")
                nc.tensor.matmul(p4[:, :cs], lhsT=w4_s[:], rhs=h3[:, :cs],
                                 start=True, stop=True)
                ob = wpool.tile([OUTD, NCHUNK], f32, tag="ob")
                nc.vector.tensor_scalar(
                    out=ob[:, :cs], in0=p4[:, :cs],
                    scalar1=b4_s[0:1, 0:1], scalar2=None, op0=OP.add)
                nc.sync.dma_start(out_d[c0:c0 + cs, :], ob[0:1, :cs])

    nc.compile()
    return nc


def make_in_maps(p, inputs, edge_dt="float16"):
    np_edt = dict(float32=np.float32, float16=np.float16)[edge_dt]
    if edge_dt == "bfloat16":
        import ml_dtypes
        np_edt = ml_dtypes.bfloat16
    x = np.ascontiguousarray(np.asarray(inputs["x"]).astype(np_edt))
    maps = []
    for c in range(p.n_cores):
        maps.append({
            "x": x,
            "midx": p.idx_arr[c],
            "smat": p.smat[c].astype(np_edt, copy=False),
            "sdiag": p.sdiag[c].astype(np_edt, copy=False),
            "xown": x[c * p.npc:(c + 1) * p.npc],
            "w1": np.asarray(inputs["W1"], dtype=np.float32),
            "b1": np.asarray(inputs["b1"], dtype=np.float32).reshape(-1, 1),
            "w2": np.asarray(inputs["W2"], dtype=np.float32),
            "b2": np.asarray(inputs["b2"], dtype=np.float32).reshape(-1, 1),
            "w3": np.asarray(inputs["W3"], dtype=np.float32),
            "b3": np.asarray(inputs["b3"], dtype=np.float32).reshape(-1, 1),
            "w4": np.asarray(inputs["W4"], dtype=np.float32),
            "b4": np.asarray(inputs["b4"], dtype=np.float32).reshape(-1, 1),
        })
    return maps


_CACHE = {}


def kernel(_trace=False, **inputs):
    from concourse.bass_utils import run_bass_kernel_spmd

    edge_index = np.asarray(inputs["edge_index"])
    p = make_plan(edge_index)
    key = (p.S, tuple(int(c[3]) for c in p.calls))
    if key not in _CACHE:
        _CACHE[key] = build_program(p)
    nc = _CACHE[key]
    res = run_bass_kernel_spmd(nc, make_in_maps(p, inputs),
                               core_ids=list(range(p.n_cores)),
                               trace=_trace)
    out = np.concatenate([res.results[c]["out"] for c in range(p.n_cores)],
                         axis=0)
    if _trace:
        return out, res
    return out
